# revision 8
# baseline (speedup 1.0000x reference)
"""AutoCorrelation layer kernel for 8 Trainium2 NeuronCores.

Math note: the reference's rfft/irfft pair over the zero-padded head dim
computes a circular cross-correlation; its mean over all lags collapses
analytically to (sum_d q_proj) * (sum_d k_proj) per head.  So
corr_mean[b,l] = (1/(H*L)) * sum_h (q[b,l] @ WqS + bqS)_h * (k[b,l] @ WkS + bkS)_h
with WqS = Wq.reshape(D,H,DK).sum(-1).  Everything downstream (top-6,
softmax, gather, output projection) follows the reference directly.

Distribution (v2): batch-parallel preprocessing — core i computes the
corr/top-6/gather/agg for batch i only (f32 throughout: the 6th/7th
top-k gap can be ~1e-5 so bf16 corr flips selections), then a tiny
AllGather of the per-batch agg vectors [1,256] feeds the column-sharded
output projection.  Wp is pre-cast to bf16 on the host and streamed as
16 resident SBUF tiles whose DMAs all start at t=0; the big matmuls
consume them as soon as agg arrives.  v is fetched via a 6-row indexed
gather instead of a full 1MB load.
"""
import sys

sys.path.insert(0, "/opt/trn_rl_repo")

import numpy as np
import ml_dtypes
import concourse.bass as bass
import concourse.mybir as mybir
import concourse.tile as tile
from concourse import bacc
from concourse.bass_utils import run_bass_kernel_spmd
from concourse.masks import make_identity

F32 = mybir.dt.float32
BF16 = mybir.dt.bfloat16
U32 = mybir.dt.uint32

N_CORES = 8
B, L, D, H, DK = 8, 1024, 256, 8, 32
K_TOP = 6
NSH = (L * D) // N_CORES          # 32768 output cols per core
TILE_N = 2048
N_TILES = NSH // TILE_N           # 16
SUBS = TILE_N // 512              # 4
SCALE = 1.0 / (H * L)

TRACE = False          # test harness sets this for profiled runs
LAST_RESULT = None     # stashed BassKernelResults from the last kernel() call
MODE = "dp"            # "dp": batch-parallel preproc + AllGather; "repl": replicated preproc

_CACHE = {}


def _build_nc():
    nc = bacc.Bacc("TRN2", target_bir_lowering=False, debug=False, num_devices=N_CORES)

    qt_d = nc.dram_tensor("qt", [D, L], F32, kind="ExternalInput").ap()
    kt_d = nc.dram_tensor("kt", [D, L], F32, kind="ExternalInput").ap()
    v_d = nc.dram_tensor("v", [L, D], F32, kind="ExternalInput").ap()
    wq_d = nc.dram_tensor("wq", [D, D], F32, kind="ExternalInput").ap()
    wk_d = nc.dram_tensor("wk", [D, D], F32, kind="ExternalInput").ap()
    wv_d = nc.dram_tensor("wv", [D, D], F32, kind="ExternalInput").ap()
    bq_d = nc.dram_tensor("bq", [1, D], F32, kind="ExternalInput").ap()
    bk_d = nc.dram_tensor("bk", [1, D], F32, kind="ExternalInput").ap()
    bv_d = nc.dram_tensor("bv", [1, D], F32, kind="ExternalInput").ap()
    wp_d = nc.dram_tensor("wp", [D, NSH], BF16, kind="ExternalInput").ap()
    bp_d = nc.dram_tensor("bp", [1, NSH], F32, kind="ExternalInput").ap()
    out_d = nc.dram_tensor("out", [B, NSH], BF16, kind="ExternalOutput").ap()

    with tile.TileContext(nc) as tc:
        with (
            tc.tile_pool(name="cst", bufs=1) as cst,
            tc.tile_pool(name="work", bufs=1) as work,
            tc.tile_pool(name="wpp", bufs=N_TILES) as wpp,
            tc.tile_pool(name="outp", bufs=2) as outp,
            tc.tile_pool(name="bpp", bufs=2) as bpp,
            tc.tile_pool(name="dr", bufs=1, space="DRAM") as dr,
            tc.tile_pool(name="ps_mm", bufs=1, space="PSUM") as ps_mm,
            tc.tile_pool(name="ps_tp", bufs=1, space="PSUM") as ps_tp,
            tc.tile_pool(name="ps_o", bufs=6, space="PSUM") as ps_o,
        ):
            # ---------------- phase 0: kick off all input DMAs ----------------
            # sync ring order: small weights -> qt/kt -> 16 wp tiles (16MB).
            wq_sb = cst.tile([128, 2, 256], F32)
            nc.sync.dma_start(wq_sb[:, :, :], wq_d.rearrange("(c p) d -> p c d", p=128))
            wk_sb = cst.tile([128, 2, 256], F32)
            nc.sync.dma_start(wk_sb[:, :, :], wk_d.rearrange("(c p) d -> p c d", p=128))
            wv_sb = cst.tile([128, 2, 256], F32)
            nc.sync.dma_start(wv_sb[:, :, :], wv_d.rearrange("(c p) d -> p c d", p=128))
            bq_sb = cst.tile([1, 256], F32)
            nc.sync.dma_start(bq_sb[:, :], bq_d)
            bk_sb = cst.tile([1, 256], F32)
            nc.sync.dma_start(bk_sb[:, :], bk_d)
            bv_sb = cst.tile([1, 256], F32)
            nc.sync.dma_start(bv_sb[:, :], bv_d)
            trq = work.tile([128, 2, L], F32)
            nc.sync.dma_start(trq[:, :, :], qt_d.rearrange("(c p) l -> p c l", p=128))
            trk = work.tile([128, 2, L], F32)
            nc.sync.dma_start(trk[:, :, :], kt_d.rearrange("(c p) l -> p c l", p=128))
            wpt = []
            for nt in range(N_TILES):
                ncol = slice(TILE_N * nt, TILE_N * (nt + 1))
                wp_t = wpp.tile([128, 2, TILE_N], BF16, tag="wp")
                eng = nc.sync if nt % 2 == 0 else nc.scalar
                eng.dma_start(
                    wp_t[:, :, :],
                    wp_d[:, ncol].rearrange("(c p) n -> p c n", p=128))
                wpt.append(wp_t)

            # ---------------- small constants ----------------
            ident8 = cst.tile([8, 8], F32)
            make_identity(nc, ident8[:, :])
            one1 = cst.tile([1, 1], F32)
            nc.vector.memset(one1[:, :], 1.0)
            sones = cst.tile([8, 1], F32)
            nc.vector.memset(sones[:, :], SCALE)

            # PE warm-up: the HAM clock gate needs ~3.4us of sustained PE
            # activity to lift the 1.2GHz cold throttle; burn it on junk
            # matmuls while the qt/kt DMAs are still in flight.
            ps_warm = ps_mm.tile([128, 512], F32, tag="mm")
            for _ in range(4):
                nc.tensor.matmul(ps_warm[:, 0:256], wq_sb[:, 0, 0:128], wq_sb[:, 0, :],
                                 start=True, stop=True)

            # head-sums of projection weights: WqS[d, h] = sum_z Wq[d, h*32+z]
            wqs = cst.tile([128, 2, 8], F32)
            nc.vector.reduce_sum(out=wqs[:, :, :],
                                 in_=wq_sb[:, :, :].rearrange("p c (h z) -> p c h z", z=DK),
                                 axis=mybir.AxisListType.X)
            wks = cst.tile([128, 2, 8], F32)
            nc.vector.reduce_sum(out=wks[:, :, :],
                                 in_=wk_sb[:, :, :].rearrange("p c (h z) -> p c h z", z=DK),
                                 axis=mybir.AxisListType.X)
            bqs_row = cst.tile([1, 8], F32)
            nc.vector.reduce_sum(out=bqs_row[:, :],
                                 in_=bq_sb[:, :].rearrange("o (h z) -> o h z", z=DK),
                                 axis=mybir.AxisListType.X)
            bks_row = cst.tile([1, 8], F32)
            nc.vector.reduce_sum(out=bks_row[:, :],
                                 in_=bk_sb[:, :].rearrange("o (h z) -> o h z", z=DK),
                                 axis=mybir.AxisListType.X)
            # [1,8] -> [8,1] via K=1 matmul against [1,1] ones
            bqs_ps = ps_tp.tile([8, 1], F32, tag="tp")
            nc.tensor.matmul(bqs_ps[:, :], bqs_row[:, :], one1[:, :], start=True, stop=True)
            bqs_vert = cst.tile([8, 1], F32)
            nc.vector.tensor_copy(bqs_vert[:, :], bqs_ps[:, :])
            bks_ps = ps_tp.tile([8, 1], F32, tag="tp")
            nc.tensor.matmul(bks_ps[:, :], bks_row[:, :], one1[:, :], start=True, stop=True)
            bks_vert = cst.tile([8, 1], F32)
            nc.vector.tensor_copy(bks_vert[:, :], bks_ps[:, :])

            # ---------------- per-batch corr (this core's batch only) ----------------
            xsT = {}
            for (tr, w_sum, bias_v, nm) in (
                (trq, wqs, bqs_vert, "q"),
                (trk, wks, bks_vert, "k"),
            ):
                xs = work.tile([8, L], F32, tag=f"{nm}sT")
                for half in range(2):
                    sl = slice(512 * half, 512 * (half + 1))
                    ps_x = ps_mm.tile([8, 512], F32, tag="mm")
                    nc.tensor.matmul(ps_x[:, :], w_sum[:, 0, :], tr[:, 0, sl], start=True, stop=False)
                    nc.tensor.matmul(ps_x[:, :], w_sum[:, 1, :], tr[:, 1, sl], start=False, stop=True)
                    nc.vector.tensor_scalar(
                        out=xs[:, sl], in0=ps_x[:, :],
                        scalar1=bias_v[:, 0:1], scalar2=None, op0=mybir.AluOpType.add)
                xsT[nm] = xs

            prod = work.tile([8, L], F32, tag="prod")
            nc.vector.tensor_mul(prod[:, :], xsT["q"][:, :], xsT["k"][:, :])
            r_sb = work.tile([1, L], F32, tag="qsT")
            for half in range(2):
                sl = slice(512 * half, 512 * (half + 1))
                ps_r = ps_mm.tile([1, 512], F32, tag="mm")
                nc.tensor.matmul(ps_r[:, :], sones[:, :], prod[:, sl], start=True, stop=True)
                nc.vector.tensor_copy(r_sb[:, sl], ps_r[:, :])

            # ---------------- top-6, softmax, 6-row gather of v ----------------
            topv = work.tile([1, 8], F32)
            nc.vector.max(topv[:, :], r_sb[:, :])
            topi = work.tile([1, 8], U32)
            nc.vector.max_index(topi[:, :], topv[:, :], r_sb[:, :])
            negm0 = work.tile([1, 1], F32)
            nc.vector.tensor_scalar_mul(negm0[:, :], topv[:, 0:1], -1.0)
            e_sb = work.tile([1, K_TOP], F32)
            nc.scalar.activation(e_sb[:, :], topv[:, 0:K_TOP],
                                 mybir.ActivationFunctionType.Exp,
                                 bias=negm0[:, 0:1], scale=1.0)
            z_sb = work.tile([1, 1], F32)
            nc.vector.reduce_sum(out=z_sb[:, :], in_=e_sb[:, :], axis=mybir.AxisListType.X)
            zinv = work.tile([1, 1], F32)
            nc.vector.reciprocal(zinv[:, :], z_sb[:, :])
            w_sb = work.tile([1, K_TOP], F32)
            nc.vector.tensor_scalar_mul(w_sb[:, :], e_sb[:, :], zinv[:, 0:1])

            # indices/weights -> columns via K=1 matmuls
            topi_f = work.tile([1, 8], F32)
            nc.vector.tensor_copy(topi_f[:, :], topi[:, :])
            idx_ps = ps_tp.tile([8, 1], F32, tag="tp")
            nc.tensor.matmul(idx_ps[:, :], topi_f[:, :], one1[:, :], start=True, stop=True)
            idx_colf = work.tile([8, 1], F32)
            nc.vector.tensor_copy(idx_colf[:, :], idx_ps[:, :])
            idx_col = work.tile([8, 1], U32)
            nc.vector.tensor_copy(idx_col[:, :], idx_colf[:, :])
            w_ps = ps_tp.tile([K_TOP, 1], F32, tag="tp")
            nc.tensor.matmul(w_ps[:, :], w_sb[:, :], one1[:, :], start=True, stop=True)
            w_col = work.tile([K_TOP, 1], F32)
            nc.vector.tensor_copy(w_col[:, :], w_ps[:, :])

            vrows = work.tile([K_TOP, 256], F32)
            nc.gpsimd.indirect_dma_start(
                out=vrows[:, :],
                out_offset=None,
                in_=v_d[:, :],
                in_offset=bass.IndirectOffsetOnAxis(ap=idx_col[0:K_TOP, 0:1], axis=0),
            )

            # vbar[1, 256] = w^T @ vrows
            vb_ps = ps_tp.tile([1, 256], F32, tag="tp")
            nc.tensor.matmul(vb_ps[:, :], w_col[:, :], vrows[:, :], start=True, stop=True)
            vbar = work.tile([1, 256], F32)
            nc.vector.tensor_copy(vbar[:, :], vb_ps[:, :])
            # vbar -> [128, 2] column chunks
            vbarT = work.tile([128, 2], F32)
            for m in range(2):
                pv = ps_tp.tile([128, 1], F32, tag="tp")
                nc.tensor.matmul(pv[:, :], vbar[0:1, 128 * m:128 * (m + 1)], one1[:, :],
                                 start=True, stop=True)
                nc.vector.tensor_copy(vbarT[:, m:m + 1], pv[:, :])

            # agg[d'] = sum_e Wv[e, d'] vbar[e] + bv[d']  -> [128, 2] (d' chunks)
            agg_sb = work.tile([128, 2], F32)
            for m in range(2):
                pa = ps_tp.tile([128, 1], F32, tag="tp")
                nc.tensor.matmul(pa[:, :], wv_sb[:, 0, 128 * m:128 * (m + 1)],
                                 vbarT[:, 0:1], start=True, stop=False)
                nc.tensor.matmul(pa[:, :], wv_sb[:, 1, 128 * m:128 * (m + 1)],
                                 vbarT[:, 1:2], start=False, stop=False)
                nc.tensor.matmul(pa[:, :], bv_sb[0:1, 128 * m:128 * (m + 1)],
                                 one1[:, :], start=False, stop=True)
                nc.vector.tensor_copy(agg_sb[:, m:m + 1], pa[:, :])

            # ---------------- AllGather agg -> [8, 256] ----------------
            agg_in = dr.tile([1, D], F32)
            nc.gpsimd.dma_start(
                agg_in[:, :].rearrange("o (m e) -> (o e) m", e=128), agg_sb[:, :])
            agg_out = dr.tile([B, D], F32)
            nc.gpsimd.collective_compute(
                "AllGather", mybir.AluOpType.bypass,
                replica_groups=[list(range(N_CORES))],
                ins=[agg_in[:, :].opt()], outs=[agg_out[:, :].opt()])
            aggf = cst.tile([8, 256], F32)
            nc.gpsimd.dma_start(aggf[:, :], agg_out[:, :])
            aggt_bf = cst.tile([128, 16], BF16)
            for m in range(2):
                pt = ps_tp.tile([128, 8], F32, tag="tp")
                nc.tensor.transpose(pt[:, :], aggf[0:8, 128 * m:128 * (m + 1)], ident8[:, :])
                nc.vector.tensor_copy(aggt_bf[:, 8 * m:8 * (m + 1)], pt[:, :])

            # ---------------- big output projection (column shard) ----------------
            # bp tiles ride the gpsimd ring (after the gather + collective there),
            # partition-broadcast in the DMA itself.
            for nt in range(N_TILES):
                ncol = slice(TILE_N * nt, TILE_N * (nt + 1))
                bp_rep = bpp.tile([8, TILE_N], F32, tag="bp")
                nc.gpsimd.dma_start(bp_rep[:, :], bp_d[0:1, ncol].to_broadcast((8, TILE_N)))
                o_sb = outp.tile([8, TILE_N], BF16)
                pss = []
                for s in range(SUBS):
                    ssl = slice(512 * s, 512 * (s + 1))
                    ps = ps_o.tile([8, 512], F32, tag="po")
                    nc.tensor.matmul(ps[:, :], aggt_bf[:, 0:8], wpt[nt][:, 0, ssl], start=True, stop=False)
                    pss.append(ps)
                for s in range(SUBS):
                    ssl = slice(512 * s, 512 * (s + 1))
                    nc.tensor.matmul(pss[s][:, :], aggt_bf[:, 8:16], wpt[nt][:, 1, ssl], start=False, stop=True)
                for s in range(SUBS):
                    ssl = slice(512 * s, 512 * (s + 1))
                    # fused PSUM->SBUF copy + bias add (bf16 out: 2x DVE rate)
                    nc.vector.tensor_add(o_sb[:, ssl], pss[s][:, :], bp_rep[:, ssl])
                nc.gpsimd.dma_start(out_d[:, ncol], o_sb[:, :])

    nc.finalize()
    return nc


def _build_nc_repl():
    """Replicated preprocessing: every core computes corr/top-6/agg for ALL
    8 batches (16MB redundant qt/kt read) so no cross-core collective is
    needed; the column-sharded projection starts as soon as local preproc
    finishes (~65us) instead of waiting ~90us for the AllGather."""
    nc = bacc.Bacc("TRN2", target_bir_lowering=False, debug=False, num_devices=N_CORES)

    qt_d = nc.dram_tensor("qt", [B * D, L], F32, kind="ExternalInput").ap()
    kt_d = nc.dram_tensor("kt", [B * D, L], F32, kind="ExternalInput").ap()
    v_d = nc.dram_tensor("v", [B * L, D], F32, kind="ExternalInput").ap()
    wq_d = nc.dram_tensor("wq", [D, D], F32, kind="ExternalInput").ap()
    wk_d = nc.dram_tensor("wk", [D, D], F32, kind="ExternalInput").ap()
    wv_d = nc.dram_tensor("wv", [D, D], F32, kind="ExternalInput").ap()
    bq_d = nc.dram_tensor("bq", [1, D], F32, kind="ExternalInput").ap()
    bk_d = nc.dram_tensor("bk", [1, D], F32, kind="ExternalInput").ap()
    bv_d = nc.dram_tensor("bv", [1, D], F32, kind="ExternalInput").ap()
    wp_d = nc.dram_tensor("wp", [D, NSH], BF16, kind="ExternalInput").ap()
    bp_d = nc.dram_tensor("bp", [1, NSH], F32, kind="ExternalInput").ap()
    out_d = nc.dram_tensor("out", [B, NSH], BF16, kind="ExternalOutput").ap()

    with tile.TileContext(nc) as tc:
        with (
            tc.tile_pool(name="cst", bufs=1) as cst,
            tc.tile_pool(name="work", bufs=1) as work,
            tc.tile_pool(name="trp", bufs=3) as trp,
            tc.tile_pool(name="wpp", bufs=N_TILES) as wpp,
            tc.tile_pool(name="outp", bufs=2) as outp,
            tc.tile_pool(name="bpp", bufs=2) as bpp,
            tc.tile_pool(name="ps_mm", bufs=1, space="PSUM") as ps_mm,
            tc.tile_pool(name="ps_tp", bufs=1, space="PSUM") as ps_tp,
            tc.tile_pool(name="ps_o", bufs=6, space="PSUM") as ps_o,
        ):
            # ---- phase 0: all input DMAs; qt/kt tiles split across both
            # HWDGE rings ahead of the wp stream ----
            wq_sb = cst.tile([128, 2, 256], F32)
            nc.sync.dma_start(wq_sb[:, :, :], wq_d.rearrange("(c p) d -> p c d", p=128))
            wk_sb = cst.tile([128, 2, 256], F32)
            nc.sync.dma_start(wk_sb[:, :, :], wk_d.rearrange("(c p) d -> p c d", p=128))
            wv_sb = cst.tile([128, 2, 256], F32)
            nc.sync.dma_start(wv_sb[:, :, :], wv_d.rearrange("(c p) d -> p c d", p=128))
            bq_sb = cst.tile([1, 256], F32)
            nc.sync.dma_start(bq_sb[:, :], bq_d)
            bk_sb = cst.tile([1, 256], F32)
            nc.sync.dma_start(bk_sb[:, :], bk_d)
            bv_sb = cst.tile([1, 256], F32)
            nc.sync.dma_start(bv_sb[:, :], bv_d)
            # per-batch transposed q/k tiles: sync carries q, scalar carries k
            trqs, trks = [], []
            for b in range(B):
                trq = trp.tile([128, 2, L], F32, tag="trq")
                nc.sync.dma_start(
                    trq[:, :, :],
                    qt_d[D * b:D * (b + 1), :].rearrange("(c p) l -> p c l", p=128))
                trqs.append(trq)
                trk = trp.tile([128, 2, L], F32, tag="trk")
                nc.scalar.dma_start(
                    trk[:, :, :],
                    kt_d[D * b:D * (b + 1), :].rearrange("(c p) l -> p c l", p=128))
                trks.append(trk)
            wpt = []
            for nt in range(N_TILES):
                ncol = slice(TILE_N * nt, TILE_N * (nt + 1))
                wp_t = wpp.tile([128, 2, TILE_N], BF16, tag="wp")
                eng = nc.sync if nt % 2 == 0 else nc.scalar
                eng.dma_start(
                    wp_t[:, :, :],
                    wp_d[:, ncol].rearrange("(c p) n -> p c n", p=128))
                wpt.append(wp_t)

            # ---- small constants ----
            ident8 = cst.tile([8, 8], F32)
            make_identity(nc, ident8[:, :])
            one1 = cst.tile([1, 1], F32)
            nc.vector.memset(one1[:, :], 1.0)
            sones = cst.tile([8, 1], F32)
            nc.vector.memset(sones[:, :], SCALE)

            # PE warm-up while the first qt/kt tiles are in flight
            ps_warm = ps_mm.tile([128, 512], F32, tag="mm")
            for _ in range(4):
                nc.tensor.matmul(ps_warm[:, 0:256], wq_sb[:, 0, 0:128], wq_sb[:, 0, :],
                                 start=True, stop=True)

            wqs = cst.tile([128, 2, 8], F32)
            nc.vector.reduce_sum(out=wqs[:, :, :],
                                 in_=wq_sb[:, :, :].rearrange("p c (h z) -> p c h z", z=DK),
                                 axis=mybir.AxisListType.X)
            wks = cst.tile([128, 2, 8], F32)
            nc.vector.reduce_sum(out=wks[:, :, :],
                                 in_=wk_sb[:, :, :].rearrange("p c (h z) -> p c h z", z=DK),
                                 axis=mybir.AxisListType.X)
            bqs_row = cst.tile([1, 8], F32)
            nc.vector.reduce_sum(out=bqs_row[:, :],
                                 in_=bq_sb[:, :].rearrange("o (h z) -> o h z", z=DK),
                                 axis=mybir.AxisListType.X)
            bks_row = cst.tile([1, 8], F32)
            nc.vector.reduce_sum(out=bks_row[:, :],
                                 in_=bk_sb[:, :].rearrange("o (h z) -> o h z", z=DK),
                                 axis=mybir.AxisListType.X)
            bqs_ps = ps_tp.tile([8, 1], F32, tag="tp")
            nc.tensor.matmul(bqs_ps[:, :], bqs_row[:, :], one1[:, :], start=True, stop=True)
            bqs_vert = cst.tile([8, 1], F32)
            nc.vector.tensor_copy(bqs_vert[:, :], bqs_ps[:, :])
            bks_ps = ps_tp.tile([8, 1], F32, tag="tp")
            nc.tensor.matmul(bks_ps[:, :], bks_row[:, :], one1[:, :], start=True, stop=True)
            bks_vert = cst.tile([8, 1], F32)
            nc.vector.tensor_copy(bks_vert[:, :], bks_ps[:, :])

            # ---- per-batch corr, top-6, softmax; collect idx/w columns ----
            idx48 = work.tile([48, 1], F32)
            wmat = work.tile([48, 8], F32)
            nc.vector.memset(wmat[:, :], 0.0)
            for b in range(B):
                xsT = {}
                for (tr, w_sum, bias_v, nm) in (
                    (trqs[b], wqs, bqs_vert, "q"),
                    (trks[b], wks, bks_vert, "k"),
                ):
                    xs = work.tile([8, L], F32, tag=f"{nm}sT")
                    for half in range(2):
                        sl = slice(512 * half, 512 * (half + 1))
                        ps_x = ps_mm.tile([8, 512], F32, tag="mm")
                        nc.tensor.matmul(ps_x[:, :], w_sum[:, 0, :], tr[:, 0, sl], start=True, stop=False)
                        nc.tensor.matmul(ps_x[:, :], w_sum[:, 1, :], tr[:, 1, sl], start=False, stop=True)
                        nc.vector.tensor_scalar(
                            out=xs[:, sl], in0=ps_x[:, :],
                            scalar1=bias_v[:, 0:1], scalar2=None, op0=mybir.AluOpType.add)
                    xsT[nm] = xs
                prod = work.tile([8, L], F32, tag="prod")
                nc.vector.tensor_mul(prod[:, :], xsT["q"][:, :], xsT["k"][:, :])
                r_sb = work.tile([1, L], F32, tag="qsT")
                for half in range(2):
                    sl = slice(512 * half, 512 * (half + 1))
                    ps_r = ps_mm.tile([1, 512], F32, tag="mm")
                    nc.tensor.matmul(ps_r[:, :], sones[:, :], prod[:, sl], start=True, stop=True)
                    nc.vector.tensor_copy(r_sb[:, sl], ps_r[:, :])

                topv = work.tile([1, 8], F32, tag="topv")
                nc.vector.max(topv[:, :], r_sb[:, :])
                topi = work.tile([1, 8], U32, tag="topi")
                nc.vector.max_index(topi[:, :], topv[:, :], r_sb[:, :])
                negm0 = work.tile([1, 1], F32, tag="negm0")
                nc.vector.tensor_scalar_mul(negm0[:, :], topv[:, 0:1], -1.0)
                e_sb = work.tile([1, K_TOP], F32, tag="e_sb")
                nc.scalar.activation(e_sb[:, :], topv[:, 0:K_TOP],
                                     mybir.ActivationFunctionType.Exp,
                                     bias=negm0[:, 0:1], scale=1.0)
                z_sb = work.tile([1, 1], F32, tag="z_sb")
                nc.vector.reduce_sum(out=z_sb[:, :], in_=e_sb[:, :], axis=mybir.AxisListType.X)
                zinv = work.tile([1, 1], F32, tag="zinv")
                nc.vector.reciprocal(zinv[:, :], z_sb[:, :])
                w_sb = work.tile([1, K_TOP], F32, tag="w_sb")
                nc.vector.tensor_scalar_mul(w_sb[:, :], e_sb[:, :], zinv[:, 0:1])

                # indices (+1024*b) and weights -> columns of idx48 / wmat
                topi_f = work.tile([1, 8], F32, tag="topi_f")
                nc.vector.tensor_copy(topi_f[:, :], topi[:, :])
                idx_ps = ps_tp.tile([8, 1], F32, tag="tp")
                nc.tensor.matmul(idx_ps[:, :], topi_f[:, :], one1[:, :], start=True, stop=True)
                nc.vector.tensor_scalar(
                    out=idx48[6 * b:6 * (b + 1), 0:1], in0=idx_ps[0:K_TOP, :],
                    scalar1=None, scalar2=None, op0=mybir.AluOpType.add,
                    const1=float(L * b))
                w_ps = ps_tp.tile([K_TOP, 1], F32, tag="tp")
                nc.tensor.matmul(w_ps[:, :], w_sb[:, :], one1[:, :], start=True, stop=True)
                nc.vector.tensor_copy(wmat[6 * b:6 * (b + 1), b:b + 1], w_ps[:, :])

            # ---- one 48-row gather of v, batched weighted-sum, Wv proj ----
            idx_u = work.tile([48, 1], U32)
            nc.vector.tensor_copy(idx_u[:, :], idx48[:, :])
            vrows = work.tile([48, 256], F32)
            nc.gpsimd.indirect_dma_start(
                out=vrows[:, :],
                out_offset=None,
                in_=v_d[:, :],
                in_offset=bass.IndirectOffsetOnAxis(ap=idx_u[:, 0:1], axis=0),
            )
            # vbar[8, 256] = wmat.T @ vrows  (block-diagonal weights)
            ps_vb = ps_mm.tile([8, 512], F32, tag="mm")
            nc.tensor.matmul(ps_vb[:, 0:256], wmat[:, :], vrows[:, :], start=True, stop=True)
            vbar = work.tile([8, 256], F32)
            nc.vector.tensor_copy(vbar[:, :], ps_vb[:, 0:256])
            # vbarT [128, 2, 8]
            vbarT = work.tile([128, 2, 8], F32)
            for m in range(2):
                pt = ps_tp.tile([128, 8], F32, tag="tp")
                nc.tensor.transpose(pt[:, :], vbar[0:8, 128 * m:128 * (m + 1)], ident8[:, :])
                nc.vector.tensor_copy(vbarT[:, m, :], pt[:, :])
            # aggf[8, 256] = vbar @ Wv + bv
            ps_a = ps_mm.tile([8, 512], F32, tag="mm")
            nc.tensor.matmul(ps_a[:, 0:256], vbarT[:, 0, :], wv_sb[:, 0, :], start=True, stop=False)
            nc.tensor.matmul(ps_a[:, 0:256], vbarT[:, 1, :], wv_sb[:, 1, :], start=False, stop=False)
            ones8m = cst.tile([1, 8], F32)
            nc.vector.memset(ones8m[:, :], 1.0)
            nc.tensor.matmul(ps_a[:, 0:256], ones8m[:, :], bv_sb[:, :], start=False, stop=True)
            aggf = cst.tile([8, 256], F32)
            nc.vector.tensor_copy(aggf[:, :], ps_a[:, 0:256])
            aggt_bf = cst.tile([128, 16], BF16)
            for m in range(2):
                pt2 = ps_tp.tile([128, 8], F32, tag="tp")
                nc.tensor.transpose(pt2[:, :], aggf[0:8, 128 * m:128 * (m + 1)], ident8[:, :])
                nc.vector.tensor_copy(aggt_bf[:, 8 * m:8 * (m + 1)], pt2[:, :])

            # ---- big output projection ----
            for nt in range(N_TILES):
                ncol = slice(TILE_N * nt, TILE_N * (nt + 1))
                bp_rep = bpp.tile([8, TILE_N], F32, tag="bp")
                nc.gpsimd.dma_start(bp_rep[:, :], bp_d[0:1, ncol].to_broadcast((8, TILE_N)))
                o_sb = outp.tile([8, TILE_N], BF16)
                pss = []
                for s in range(SUBS):
                    ssl = slice(512 * s, 512 * (s + 1))
                    ps = ps_o.tile([8, 512], F32, tag="po")
                    nc.tensor.matmul(ps[:, :], aggt_bf[:, 0:8], wpt[nt][:, 0, ssl], start=True, stop=False)
                    pss.append(ps)
                for s in range(SUBS):
                    ssl = slice(512 * s, 512 * (s + 1))
                    nc.tensor.matmul(pss[s][:, :], aggt_bf[:, 8:16], wpt[nt][:, 1, ssl], start=False, stop=True)
                for s in range(SUBS):
                    ssl = slice(512 * s, 512 * (s + 1))
                    nc.vector.tensor_add(o_sb[:, ssl], pss[s][:, :], bp_rep[:, ssl])
                nc.gpsimd.dma_start(out_d[:, ncol], o_sb[:, :])

    nc.finalize()
    return nc


def _get_nc():
    if "nc" not in _CACHE:
        _CACHE["nc"] = _build_nc_repl() if MODE == "repl" else _build_nc()
    return _CACHE["nc"]


def kernel(queries, keys, values, Wq, bq, Wk, bk, Wv, bv, Wp, bp):
    queries = np.asarray(queries, np.float32)
    keys = np.asarray(keys, np.float32)
    values = np.asarray(values, np.float32)
    Wq = np.ascontiguousarray(np.asarray(Wq, np.float32))
    Wk = np.ascontiguousarray(np.asarray(Wk, np.float32))
    Wv = np.ascontiguousarray(np.asarray(Wv, np.float32))
    bq = np.asarray(bq, np.float32).reshape(1, D)
    bk = np.asarray(bk, np.float32).reshape(1, D)
    bv = np.asarray(bv, np.float32).reshape(1, D)
    Wp = np.asarray(Wp, np.float32)
    bp = np.asarray(bp, np.float32)

    nc = _get_nc()
    qT = np.ascontiguousarray(queries.transpose(0, 2, 1))   # [B, D, L]
    kT = np.ascontiguousarray(keys.transpose(0, 2, 1))
    Wp_bf = np.asarray(Wp, dtype=ml_dtypes.bfloat16)
    in_maps = []
    for i in range(N_CORES):
        cols = slice(NSH * i, NSH * (i + 1))
        m = {
            "wq": Wq, "wk": Wk, "wv": Wv,
            "bq": bq, "bk": bk, "bv": bv,
            "wp": np.ascontiguousarray(Wp_bf[:, cols]),
            "bp": np.ascontiguousarray(bp[cols]).reshape(1, NSH),
        }
        if MODE == "repl":
            m.update({"qt": qT.reshape(B * D, L), "kt": kT.reshape(B * D, L),
                      "v": values.reshape(B * L, D)})
        else:
            m.update({"qt": qT[i], "kt": kT[i], "v": values[i]})
        in_maps.append(m)
    res = run_bass_kernel_spmd(nc, in_maps, core_ids=list(range(N_CORES)), trace=TRACE)
    global LAST_RESULT
    LAST_RESULT = res
    out = np.concatenate([np.asarray(res.results[i]["out"], np.float32) for i in range(N_CORES)], axis=1)
    return out.reshape(B, L, D)


# revision 11
# speedup vs baseline: 1.0048x; 1.0048x over previous
"""AutoCorrelation layer kernel for 8 Trainium2 NeuronCores.

Math note: the reference's rfft/irfft pair over the zero-padded head dim
computes a circular cross-correlation; its mean over all lags collapses
analytically to (sum_d q_proj) * (sum_d k_proj) per head.  So
corr_mean[b,l] = (1/(H*L)) * sum_h (q[b,l] @ WqS + bqS)_h * (k[b,l] @ WkS + bkS)_h
with WqS = Wq.reshape(D,H,DK).sum(-1).  Everything downstream (top-6,
softmax, gather, output projection) follows the reference directly.

Distribution (v2): batch-parallel preprocessing — core i computes the
corr/top-6/gather/agg for batch i only (f32 throughout: the 6th/7th
top-k gap can be ~1e-5 so bf16 corr flips selections), then a tiny
AllGather of the per-batch agg vectors [1,256] feeds the column-sharded
output projection.  Wp is pre-cast to bf16 on the host and streamed as
16 resident SBUF tiles whose DMAs all start at t=0; the big matmuls
consume them as soon as agg arrives.  v is fetched via a 6-row indexed
gather instead of a full 1MB load.
"""
import sys

sys.path.insert(0, "/opt/trn_rl_repo")

import numpy as np
import ml_dtypes
import concourse.bass as bass
import concourse.mybir as mybir
import concourse.tile as tile
from concourse import bacc
from concourse.bass_utils import run_bass_kernel_spmd
from concourse.masks import make_identity

F32 = mybir.dt.float32
BF16 = mybir.dt.bfloat16
U32 = mybir.dt.uint32

N_CORES = 8
B, L, D, H, DK = 8, 1024, 256, 8, 32
K_TOP = 6
NSH = (L * D) // N_CORES          # 32768 output cols per core
TILE_N = 2048
N_TILES = NSH // TILE_N           # 16
SUBS = TILE_N // 512              # 4
SCALE = 1.0 / (H * L)

TRACE = False          # test harness sets this for profiled runs
LAST_RESULT = None     # stashed BassKernelResults from the last kernel() call
MODE = "dp"            # "dp": batch-parallel preproc + AllGather; "repl": replicated preproc

_CACHE = {}


def _build_nc():
    nc = bacc.Bacc("TRN2", target_bir_lowering=False, debug=False, num_devices=N_CORES)

    qt_d = nc.dram_tensor("qt", [D, L], F32, kind="ExternalInput").ap()
    kt_d = nc.dram_tensor("kt", [D, L], F32, kind="ExternalInput").ap()
    v_d = nc.dram_tensor("v", [L, D], F32, kind="ExternalInput").ap()
    wq_d = nc.dram_tensor("wq", [D, D], F32, kind="ExternalInput").ap()
    wk_d = nc.dram_tensor("wk", [D, D], F32, kind="ExternalInput").ap()
    wv_d = nc.dram_tensor("wv", [D, D], F32, kind="ExternalInput").ap()
    bq_d = nc.dram_tensor("bq", [1, D], F32, kind="ExternalInput").ap()
    bk_d = nc.dram_tensor("bk", [1, D], F32, kind="ExternalInput").ap()
    bv_d = nc.dram_tensor("bv", [1, D], F32, kind="ExternalInput").ap()
    wp_d = nc.dram_tensor("wp", [D, NSH], BF16, kind="ExternalInput").ap()
    bp_d = nc.dram_tensor("bp", [1, 4 * 128 * 64 * 8], BF16, kind="ExternalInput").ap()
    out_d = nc.dram_tensor("out", [4 * 128, 64 * 8], BF16, kind="ExternalOutput").ap()

    with tile.TileContext(nc) as tc:
        with (
            tc.tile_pool(name="cst", bufs=1) as cst,
            tc.tile_pool(name="work", bufs=1) as work,
            tc.tile_pool(name="wpp", bufs=N_TILES) as wpp,
            tc.tile_pool(name="outp", bufs=2) as outp,
            tc.tile_pool(name="bpp", bufs=2) as bpp,
            tc.tile_pool(name="dr", bufs=1, space="DRAM") as dr,
            tc.tile_pool(name="ps_mm", bufs=1, space="PSUM") as ps_mm,
            tc.tile_pool(name="ps_tp", bufs=1, space="PSUM") as ps_tp,
            tc.tile_pool(name="ps_o", bufs=6, space="PSUM") as ps_o,
        ):
            # ---------------- phase 0: kick off all input DMAs ----------------
            # sync ring order: small weights -> qt/kt -> 16 wp tiles (16MB).
            wq_sb = cst.tile([128, 2, 256], F32)
            nc.sync.dma_start(wq_sb[:, :, :], wq_d.rearrange("(c p) d -> p c d", p=128))
            wk_sb = cst.tile([128, 2, 256], F32)
            nc.sync.dma_start(wk_sb[:, :, :], wk_d.rearrange("(c p) d -> p c d", p=128))
            wv_sb = cst.tile([128, 2, 256], F32)
            nc.sync.dma_start(wv_sb[:, :, :], wv_d.rearrange("(c p) d -> p c d", p=128))
            bq_sb = cst.tile([1, 256], F32)
            nc.sync.dma_start(bq_sb[:, :], bq_d)
            bk_sb = cst.tile([1, 256], F32)
            nc.sync.dma_start(bk_sb[:, :], bk_d)
            bv_sb = cst.tile([1, 256], F32)
            nc.sync.dma_start(bv_sb[:, :], bv_d)
            trq = work.tile([128, 2, L], F32)
            nc.sync.dma_start(trq[:, :, :], qt_d.rearrange("(c p) l -> p c l", p=128))
            trk = work.tile([128, 2, L], F32)
            nc.sync.dma_start(trk[:, :, :], kt_d.rearrange("(c p) l -> p c l", p=128))
            wpt = []
            for nt in range(N_TILES):
                ncol = slice(TILE_N * nt, TILE_N * (nt + 1))
                wp_t = wpp.tile([128, 2, TILE_N], BF16, tag="wp")
                eng = nc.sync if nt % 2 == 0 else nc.scalar
                eng.dma_start(
                    wp_t[:, :, :],
                    wp_d[:, ncol].rearrange("(c p) n -> p c n", p=128))
                wpt.append(wp_t)

            # ---------------- small constants ----------------
            ident8 = cst.tile([8, 8], F32)
            make_identity(nc, ident8[:, :])
            one1 = cst.tile([1, 1], F32)
            nc.vector.memset(one1[:, :], 1.0)
            sones = cst.tile([8, 1], F32)
            nc.vector.memset(sones[:, :], SCALE)

            # PE warm-up: the HAM clock gate needs ~3.4us of sustained PE
            # activity to lift the 1.2GHz cold throttle; burn it on junk
            # matmuls while the qt/kt DMAs are still in flight.
            ps_warm = ps_mm.tile([128, 512], F32, tag="mm")
            for _ in range(4):
                nc.tensor.matmul(ps_warm[:, 0:256], wq_sb[:, 0, 0:128], wq_sb[:, 0, :],
                                 start=True, stop=True)

            # head-sums of projection weights: WqS[d, h] = sum_z Wq[d, h*32+z]
            wqs = cst.tile([128, 2, 8], F32)
            nc.vector.reduce_sum(out=wqs[:, :, :],
                                 in_=wq_sb[:, :, :].rearrange("p c (h z) -> p c h z", z=DK),
                                 axis=mybir.AxisListType.X)
            wks = cst.tile([128, 2, 8], F32)
            nc.vector.reduce_sum(out=wks[:, :, :],
                                 in_=wk_sb[:, :, :].rearrange("p c (h z) -> p c h z", z=DK),
                                 axis=mybir.AxisListType.X)
            bqs_row = cst.tile([1, 8], F32)
            nc.vector.reduce_sum(out=bqs_row[:, :],
                                 in_=bq_sb[:, :].rearrange("o (h z) -> o h z", z=DK),
                                 axis=mybir.AxisListType.X)
            bks_row = cst.tile([1, 8], F32)
            nc.vector.reduce_sum(out=bks_row[:, :],
                                 in_=bk_sb[:, :].rearrange("o (h z) -> o h z", z=DK),
                                 axis=mybir.AxisListType.X)
            # [1,8] -> [8,1] via K=1 matmul against [1,1] ones
            bqs_ps = ps_tp.tile([8, 1], F32, tag="tp")
            nc.tensor.matmul(bqs_ps[:, :], bqs_row[:, :], one1[:, :], start=True, stop=True)
            bqs_vert = cst.tile([8, 1], F32)
            nc.vector.tensor_copy(bqs_vert[:, :], bqs_ps[:, :])
            bks_ps = ps_tp.tile([8, 1], F32, tag="tp")
            nc.tensor.matmul(bks_ps[:, :], bks_row[:, :], one1[:, :], start=True, stop=True)
            bks_vert = cst.tile([8, 1], F32)
            nc.vector.tensor_copy(bks_vert[:, :], bks_ps[:, :])

            # ---------------- per-batch corr (this core's batch only) ----------------
            xsT = {}
            for (tr, w_sum, bias_v, nm) in (
                (trq, wqs, bqs_vert, "q"),
                (trk, wks, bks_vert, "k"),
            ):
                xs = work.tile([8, L], F32, tag=f"{nm}sT")
                for half in range(2):
                    sl = slice(512 * half, 512 * (half + 1))
                    ps_x = ps_mm.tile([8, 512], F32, tag="mm")
                    nc.tensor.matmul(ps_x[:, :], w_sum[:, 0, :], tr[:, 0, sl], start=True, stop=False)
                    nc.tensor.matmul(ps_x[:, :], w_sum[:, 1, :], tr[:, 1, sl], start=False, stop=True)
                    nc.vector.tensor_scalar(
                        out=xs[:, sl], in0=ps_x[:, :],
                        scalar1=bias_v[:, 0:1], scalar2=None, op0=mybir.AluOpType.add)
                xsT[nm] = xs

            prod = work.tile([8, L], F32, tag="prod")
            nc.vector.tensor_mul(prod[:, :], xsT["q"][:, :], xsT["k"][:, :])
            r_sb = work.tile([1, L], F32, tag="qsT")
            for half in range(2):
                sl = slice(512 * half, 512 * (half + 1))
                ps_r = ps_mm.tile([1, 512], F32, tag="mm")
                nc.tensor.matmul(ps_r[:, :], sones[:, :], prod[:, sl], start=True, stop=True)
                nc.vector.tensor_copy(r_sb[:, sl], ps_r[:, :])

            # ---------------- top-6, softmax, 6-row gather of v ----------------
            topv = work.tile([1, 8], F32)
            nc.vector.max(topv[:, :], r_sb[:, :])
            topi = work.tile([1, 8], U32)
            nc.vector.max_index(topi[:, :], topv[:, :], r_sb[:, :])
            negm0 = work.tile([1, 1], F32)
            nc.vector.tensor_scalar_mul(negm0[:, :], topv[:, 0:1], -1.0)
            e_sb = work.tile([1, K_TOP], F32)
            nc.scalar.activation(e_sb[:, :], topv[:, 0:K_TOP],
                                 mybir.ActivationFunctionType.Exp,
                                 bias=negm0[:, 0:1], scale=1.0)
            z_sb = work.tile([1, 1], F32)
            nc.vector.reduce_sum(out=z_sb[:, :], in_=e_sb[:, :], axis=mybir.AxisListType.X)
            zinv = work.tile([1, 1], F32)
            nc.vector.reciprocal(zinv[:, :], z_sb[:, :])
            w_sb = work.tile([1, K_TOP], F32)
            nc.vector.tensor_scalar_mul(w_sb[:, :], e_sb[:, :], zinv[:, 0:1])

            # indices/weights -> columns via K=1 matmuls
            topi_f = work.tile([1, 8], F32)
            nc.vector.tensor_copy(topi_f[:, :], topi[:, :])
            idx_ps = ps_tp.tile([8, 1], F32, tag="tp")
            nc.tensor.matmul(idx_ps[:, :], topi_f[:, :], one1[:, :], start=True, stop=True)
            idx_colf = work.tile([8, 1], F32)
            nc.vector.tensor_copy(idx_colf[:, :], idx_ps[:, :])
            idx_col = work.tile([8, 1], U32)
            nc.vector.tensor_copy(idx_col[:, :], idx_colf[:, :])
            w_ps = ps_tp.tile([K_TOP, 1], F32, tag="tp")
            nc.tensor.matmul(w_ps[:, :], w_sb[:, :], one1[:, :], start=True, stop=True)
            w_col = work.tile([K_TOP, 1], F32)
            nc.vector.tensor_copy(w_col[:, :], w_ps[:, :])

            vrows = work.tile([K_TOP, 256], F32)
            nc.gpsimd.indirect_dma_start(
                out=vrows[:, :],
                out_offset=None,
                in_=v_d[:, :],
                in_offset=bass.IndirectOffsetOnAxis(ap=idx_col[0:K_TOP, 0:1], axis=0),
            )

            # vbar[1, 256] = w^T @ vrows
            vb_ps = ps_tp.tile([1, 256], F32, tag="tp")
            nc.tensor.matmul(vb_ps[:, :], w_col[:, :], vrows[:, :], start=True, stop=True)
            vbar = work.tile([1, 256], F32)
            nc.vector.tensor_copy(vbar[:, :], vb_ps[:, :])
            # vbar -> [128, 2] column chunks
            vbarT = work.tile([128, 2], F32)
            for m in range(2):
                pv = ps_tp.tile([128, 1], F32, tag="tp")
                nc.tensor.matmul(pv[:, :], vbar[0:1, 128 * m:128 * (m + 1)], one1[:, :],
                                 start=True, stop=True)
                nc.vector.tensor_copy(vbarT[:, m:m + 1], pv[:, :])

            # agg[d'] = sum_e Wv[e, d'] vbar[e] + bv[d']  -> [128, 2] (d' chunks)
            agg_sb = work.tile([128, 2], F32)
            for m in range(2):
                pa = ps_tp.tile([128, 1], F32, tag="tp")
                nc.tensor.matmul(pa[:, :], wv_sb[:, 0, 128 * m:128 * (m + 1)],
                                 vbarT[:, 0:1], start=True, stop=False)
                nc.tensor.matmul(pa[:, :], wv_sb[:, 1, 128 * m:128 * (m + 1)],
                                 vbarT[:, 1:2], start=False, stop=False)
                nc.tensor.matmul(pa[:, :], bv_sb[0:1, 128 * m:128 * (m + 1)],
                                 one1[:, :], start=False, stop=True)
                nc.vector.tensor_copy(agg_sb[:, m:m + 1], pa[:, :])

            # ---------------- AllGather agg -> [8, 256] ----------------
            agg_in = dr.tile([1, D], F32)
            nc.gpsimd.dma_start(
                agg_in[:, :].rearrange("o (m e) -> (o e) m", e=128), agg_sb[:, :])
            agg_out = dr.tile([B, D], F32)
            nc.gpsimd.collective_compute(
                "AllGather", mybir.AluOpType.bypass,
                replica_groups=[list(range(N_CORES))],
                ins=[agg_in[:, :].opt()], outs=[agg_out[:, :].opt()])
            aggf = cst.tile([8, 256], F32)
            nc.gpsimd.dma_start(aggf[:, :], agg_out[:, :])
            aggt_bf = cst.tile([128, 16], BF16)
            for m in range(2):
                pt = ps_tp.tile([128, 8], F32, tag="tp")
                nc.tensor.transpose(pt[:, :], aggf[0:8, 128 * m:128 * (m + 1)], ident8[:, :])
                nc.vector.tensor_copy(aggt_bf[:, 8 * m:8 * (m + 1)], pt[:, :])

            # ---------------- big output projection, transposed ----------------
            # outT[n, b] = sum_k Wp[k, n] agg[b, k]: Wp chunks are the
            # STATIONARY operand (M=128), aggt streams (N=8).  PSUM fills a
            # whole bank [128, 64, 8] before one full-width DVE drain that
            # also adds the (host-scrambled, pre-replicated) bias.  Host
            # unscrambles the [4, 128, 64, 8] output layout.
            bp_sb = cst.tile([128, 4, 64, 8], BF16)
            nc.sync.dma_start(bp_sb[:, :, :, :], bp_d.rearrange("o (t p c b) -> (o p) t c b", t=4, p=128, c=64))
            for t in range(4):
                ps = ps_o.tile([128, 64, 8], F32, tag="po")
                for c in range(64):
                    n0 = 8192 * t + 128 * c
                    wt = wpt[n0 // TILE_N]
                    co = n0 % TILE_N
                    nc.tensor.matmul(ps[:, c, :], wt[:, 0, co:co + 128], aggt_bf[:, 0:8],
                                     start=True, stop=False)
                    nc.tensor.matmul(ps[:, c, :], wt[:, 1, co:co + 128], aggt_bf[:, 8:16],
                                     start=False, stop=True)
                o_sbT = outp.tile([128, 64, 8], BF16)
                nc.vector.tensor_add(o_sbT[:, :, :], ps[:, :, :], bp_sb[:, t, :, :])
                nc.gpsimd.dma_start(out_d[128 * t:128 * (t + 1), :], o_sbT[:, :, :].rearrange("p c b -> p (c b)"))

    nc.finalize()
    return nc


def _build_nc_repl():
    """Replicated preprocessing: every core computes corr/top-6/agg for ALL
    8 batches (16MB redundant qt/kt read) so no cross-core collective is
    needed; the column-sharded projection starts as soon as local preproc
    finishes (~65us) instead of waiting ~90us for the AllGather."""
    nc = bacc.Bacc("TRN2", target_bir_lowering=False, debug=False, num_devices=N_CORES)

    qt_d = nc.dram_tensor("qt", [B * D, L], F32, kind="ExternalInput").ap()
    kt_d = nc.dram_tensor("kt", [B * D, L], F32, kind="ExternalInput").ap()
    v_d = nc.dram_tensor("v", [B * L, D], F32, kind="ExternalInput").ap()
    wq_d = nc.dram_tensor("wq", [D, D], F32, kind="ExternalInput").ap()
    wk_d = nc.dram_tensor("wk", [D, D], F32, kind="ExternalInput").ap()
    wv_d = nc.dram_tensor("wv", [D, D], F32, kind="ExternalInput").ap()
    bq_d = nc.dram_tensor("bq", [1, D], F32, kind="ExternalInput").ap()
    bk_d = nc.dram_tensor("bk", [1, D], F32, kind="ExternalInput").ap()
    bv_d = nc.dram_tensor("bv", [1, D], F32, kind="ExternalInput").ap()
    wp_d = nc.dram_tensor("wp", [D, NSH], BF16, kind="ExternalInput").ap()
    bp_d = nc.dram_tensor("bp", [1, NSH], BF16, kind="ExternalInput").ap()
    out_d = nc.dram_tensor("out", [B, NSH], BF16, kind="ExternalOutput").ap()

    with tile.TileContext(nc) as tc:
        with (
            tc.tile_pool(name="cst", bufs=1) as cst,
            tc.tile_pool(name="work", bufs=1) as work,
            tc.tile_pool(name="trp", bufs=3) as trp,
            tc.tile_pool(name="wpp", bufs=N_TILES) as wpp,
            tc.tile_pool(name="outp", bufs=2) as outp,
            tc.tile_pool(name="bpp", bufs=2) as bpp,
            tc.tile_pool(name="ps_mm", bufs=1, space="PSUM") as ps_mm,
            tc.tile_pool(name="ps_tp", bufs=1, space="PSUM") as ps_tp,
            tc.tile_pool(name="ps_o", bufs=6, space="PSUM") as ps_o,
        ):
            # ---- phase 0: all input DMAs; qt/kt tiles split across both
            # HWDGE rings ahead of the wp stream ----
            wq_sb = cst.tile([128, 2, 256], F32)
            nc.sync.dma_start(wq_sb[:, :, :], wq_d.rearrange("(c p) d -> p c d", p=128))
            wk_sb = cst.tile([128, 2, 256], F32)
            nc.sync.dma_start(wk_sb[:, :, :], wk_d.rearrange("(c p) d -> p c d", p=128))
            wv_sb = cst.tile([128, 2, 256], F32)
            nc.sync.dma_start(wv_sb[:, :, :], wv_d.rearrange("(c p) d -> p c d", p=128))
            bq_sb = cst.tile([1, 256], F32)
            nc.sync.dma_start(bq_sb[:, :], bq_d)
            bk_sb = cst.tile([1, 256], F32)
            nc.sync.dma_start(bk_sb[:, :], bk_d)
            bv_sb = cst.tile([1, 256], F32)
            nc.sync.dma_start(bv_sb[:, :], bv_d)
            # per-batch transposed q/k tiles: sync carries q, scalar carries k
            trqs, trks = [], []
            for b in range(B):
                trq = trp.tile([128, 2, L], F32, tag="trq")
                nc.sync.dma_start(
                    trq[:, :, :],
                    qt_d[D * b:D * (b + 1), :].rearrange("(c p) l -> p c l", p=128))
                trqs.append(trq)
                trk = trp.tile([128, 2, L], F32, tag="trk")
                nc.scalar.dma_start(
                    trk[:, :, :],
                    kt_d[D * b:D * (b + 1), :].rearrange("(c p) l -> p c l", p=128))
                trks.append(trk)
            wpt = []
            for nt in range(N_TILES):
                ncol = slice(TILE_N * nt, TILE_N * (nt + 1))
                wp_t = wpp.tile([128, 2, TILE_N], BF16, tag="wp")
                eng = nc.sync if nt % 2 == 0 else nc.scalar
                eng.dma_start(
                    wp_t[:, :, :],
                    wp_d[:, ncol].rearrange("(c p) n -> p c n", p=128))
                wpt.append(wp_t)

            # ---- small constants ----
            ident8 = cst.tile([8, 8], F32)
            make_identity(nc, ident8[:, :])
            one1 = cst.tile([1, 1], F32)
            nc.vector.memset(one1[:, :], 1.0)
            sones = cst.tile([8, 1], F32)
            nc.vector.memset(sones[:, :], SCALE)

            # PE warm-up while the first qt/kt tiles are in flight
            ps_warm = ps_mm.tile([128, 512], F32, tag="mm")
            for _ in range(4):
                nc.tensor.matmul(ps_warm[:, 0:256], wq_sb[:, 0, 0:128], wq_sb[:, 0, :],
                                 start=True, stop=True)

            wqs = cst.tile([128, 2, 8], F32)
            nc.vector.reduce_sum(out=wqs[:, :, :],
                                 in_=wq_sb[:, :, :].rearrange("p c (h z) -> p c h z", z=DK),
                                 axis=mybir.AxisListType.X)
            wks = cst.tile([128, 2, 8], F32)
            nc.vector.reduce_sum(out=wks[:, :, :],
                                 in_=wk_sb[:, :, :].rearrange("p c (h z) -> p c h z", z=DK),
                                 axis=mybir.AxisListType.X)
            bqs_row = cst.tile([1, 8], F32)
            nc.vector.reduce_sum(out=bqs_row[:, :],
                                 in_=bq_sb[:, :].rearrange("o (h z) -> o h z", z=DK),
                                 axis=mybir.AxisListType.X)
            bks_row = cst.tile([1, 8], F32)
            nc.vector.reduce_sum(out=bks_row[:, :],
                                 in_=bk_sb[:, :].rearrange("o (h z) -> o h z", z=DK),
                                 axis=mybir.AxisListType.X)
            bqs_ps = ps_tp.tile([8, 1], F32, tag="tp")
            nc.tensor.matmul(bqs_ps[:, :], bqs_row[:, :], one1[:, :], start=True, stop=True)
            bqs_vert = cst.tile([8, 1], F32)
            nc.vector.tensor_copy(bqs_vert[:, :], bqs_ps[:, :])
            bks_ps = ps_tp.tile([8, 1], F32, tag="tp")
            nc.tensor.matmul(bks_ps[:, :], bks_row[:, :], one1[:, :], start=True, stop=True)
            bks_vert = cst.tile([8, 1], F32)
            nc.vector.tensor_copy(bks_vert[:, :], bks_ps[:, :])

            # ---- per-batch corr, top-6, softmax; collect idx/w columns ----
            idx48 = work.tile([48, 1], F32)
            wmat = work.tile([48, 8], F32)
            nc.vector.memset(wmat[:, :], 0.0)
            for b in range(B):
                xsT = {}
                for (tr, w_sum, bias_v, nm) in (
                    (trqs[b], wqs, bqs_vert, "q"),
                    (trks[b], wks, bks_vert, "k"),
                ):
                    xs = work.tile([8, L], F32, tag=f"{nm}sT")
                    for half in range(2):
                        sl = slice(512 * half, 512 * (half + 1))
                        ps_x = ps_mm.tile([8, 512], F32, tag="mm")
                        nc.tensor.matmul(ps_x[:, :], w_sum[:, 0, :], tr[:, 0, sl], start=True, stop=False)
                        nc.tensor.matmul(ps_x[:, :], w_sum[:, 1, :], tr[:, 1, sl], start=False, stop=True)
                        nc.vector.tensor_scalar(
                            out=xs[:, sl], in0=ps_x[:, :],
                            scalar1=bias_v[:, 0:1], scalar2=None, op0=mybir.AluOpType.add)
                    xsT[nm] = xs
                prod = work.tile([8, L], F32, tag="prod")
                nc.vector.tensor_mul(prod[:, :], xsT["q"][:, :], xsT["k"][:, :])
                r_sb = work.tile([1, L], F32, tag="qsT")
                for half in range(2):
                    sl = slice(512 * half, 512 * (half + 1))
                    ps_r = ps_mm.tile([1, 512], F32, tag="mm")
                    nc.tensor.matmul(ps_r[:, :], sones[:, :], prod[:, sl], start=True, stop=True)
                    nc.vector.tensor_copy(r_sb[:, sl], ps_r[:, :])

                topv = work.tile([1, 8], F32, tag="topv")
                nc.vector.max(topv[:, :], r_sb[:, :])
                topi = work.tile([1, 8], U32, tag="topi")
                nc.vector.max_index(topi[:, :], topv[:, :], r_sb[:, :])
                negm0 = work.tile([1, 1], F32, tag="negm0")
                nc.vector.tensor_scalar_mul(negm0[:, :], topv[:, 0:1], -1.0)
                e_sb = work.tile([1, K_TOP], F32, tag="e_sb")
                nc.scalar.activation(e_sb[:, :], topv[:, 0:K_TOP],
                                     mybir.ActivationFunctionType.Exp,
                                     bias=negm0[:, 0:1], scale=1.0)
                z_sb = work.tile([1, 1], F32, tag="z_sb")
                nc.vector.reduce_sum(out=z_sb[:, :], in_=e_sb[:, :], axis=mybir.AxisListType.X)
                zinv = work.tile([1, 1], F32, tag="zinv")
                nc.vector.reciprocal(zinv[:, :], z_sb[:, :])
                w_sb = work.tile([1, K_TOP], F32, tag="w_sb")
                nc.vector.tensor_scalar_mul(w_sb[:, :], e_sb[:, :], zinv[:, 0:1])

                # indices (+1024*b) and weights -> columns of idx48 / wmat
                topi_f = work.tile([1, 8], F32, tag="topi_f")
                nc.vector.tensor_copy(topi_f[:, :], topi[:, :])
                idx_ps = ps_tp.tile([8, 1], F32, tag="tp")
                nc.tensor.matmul(idx_ps[:, :], topi_f[:, :], one1[:, :], start=True, stop=True)
                nc.vector.tensor_scalar(
                    out=idx48[6 * b:6 * (b + 1), 0:1], in0=idx_ps[0:K_TOP, :],
                    scalar1=None, scalar2=None, op0=mybir.AluOpType.add,
                    const1=float(L * b))
                w_ps = ps_tp.tile([K_TOP, 1], F32, tag="tp")
                nc.tensor.matmul(w_ps[:, :], w_sb[:, :], one1[:, :], start=True, stop=True)
                nc.vector.tensor_copy(wmat[6 * b:6 * (b + 1), b:b + 1], w_ps[:, :])

            # ---- one 48-row gather of v, batched weighted-sum, Wv proj ----
            idx_u = work.tile([48, 1], U32)
            nc.vector.tensor_copy(idx_u[:, :], idx48[:, :])
            vrows = work.tile([48, 256], F32)
            nc.gpsimd.indirect_dma_start(
                out=vrows[:, :],
                out_offset=None,
                in_=v_d[:, :],
                in_offset=bass.IndirectOffsetOnAxis(ap=idx_u[:, 0:1], axis=0),
            )
            # vbar[8, 256] = wmat.T @ vrows  (block-diagonal weights)
            ps_vb = ps_mm.tile([8, 512], F32, tag="mm")
            nc.tensor.matmul(ps_vb[:, 0:256], wmat[:, :], vrows[:, :], start=True, stop=True)
            vbar = work.tile([8, 256], F32)
            nc.vector.tensor_copy(vbar[:, :], ps_vb[:, 0:256])
            # vbarT [128, 2, 8]
            vbarT = work.tile([128, 2, 8], F32)
            for m in range(2):
                pt = ps_tp.tile([128, 8], F32, tag="tp")
                nc.tensor.transpose(pt[:, :], vbar[0:8, 128 * m:128 * (m + 1)], ident8[:, :])
                nc.vector.tensor_copy(vbarT[:, m, :], pt[:, :])
            # aggf[8, 256] = vbar @ Wv + bv
            ps_a = ps_mm.tile([8, 512], F32, tag="mm")
            nc.tensor.matmul(ps_a[:, 0:256], vbarT[:, 0, :], wv_sb[:, 0, :], start=True, stop=False)
            nc.tensor.matmul(ps_a[:, 0:256], vbarT[:, 1, :], wv_sb[:, 1, :], start=False, stop=False)
            ones8m = cst.tile([1, 8], F32)
            nc.vector.memset(ones8m[:, :], 1.0)
            nc.tensor.matmul(ps_a[:, 0:256], ones8m[:, :], bv_sb[:, :], start=False, stop=True)
            aggf = cst.tile([8, 256], F32)
            nc.vector.tensor_copy(aggf[:, :], ps_a[:, 0:256])
            aggt_bf = cst.tile([128, 16], BF16)
            for m in range(2):
                pt2 = ps_tp.tile([128, 8], F32, tag="tp")
                nc.tensor.transpose(pt2[:, :], aggf[0:8, 128 * m:128 * (m + 1)], ident8[:, :])
                nc.vector.tensor_copy(aggt_bf[:, 8 * m:8 * (m + 1)], pt2[:, :])

            # ---- big output projection (bias as K=1 matmul row; split drains) ----
            ones8bb = cst.tile([1, 8], BF16)
            nc.vector.memset(ones8bb[:, :], 1.0)
            for nt in range(N_TILES):
                ncol = slice(TILE_N * nt, TILE_N * (nt + 1))
                bp_t = bpp.tile([1, TILE_N], BF16, tag="bp")
                nc.gpsimd.dma_start(bp_t[:, :], bp_d[0:1, ncol])
                o_sb = outp.tile([8, TILE_N], BF16)
                pss = []
                for s in range(SUBS):
                    ssl = slice(512 * s, 512 * (s + 1))
                    ps = ps_o.tile([8, 512], F32, tag="po")
                    nc.tensor.matmul(ps[:, :], aggt_bf[:, 0:8], wpt[nt][:, 0, ssl], start=True, stop=False)
                    pss.append(ps)
                for s in range(SUBS):
                    ssl = slice(512 * s, 512 * (s + 1))
                    nc.tensor.matmul(pss[s][:, :], aggt_bf[:, 8:16], wpt[nt][:, 1, ssl], start=False, stop=False)
                for s in range(SUBS):
                    ssl = slice(512 * s, 512 * (s + 1))
                    nc.tensor.matmul(pss[s][:, :], ones8bb[:, :], bp_t[:, ssl], start=False, stop=True)
                for s in range(SUBS):
                    ssl = slice(512 * s, 512 * (s + 1))
                    if s % 2 == 0:
                        nc.scalar.copy(o_sb[:, ssl], pss[s][:, :])
                    else:
                        nc.vector.tensor_copy(o_sb[:, ssl], pss[s][:, :])
                nc.gpsimd.dma_start(out_d[:, ncol], o_sb[:, :])

    nc.finalize()
    return nc


def _get_nc():
    if "nc" not in _CACHE:
        _CACHE["nc"] = _build_nc_repl() if MODE == "repl" else _build_nc()
    return _CACHE["nc"]


def kernel(queries, keys, values, Wq, bq, Wk, bk, Wv, bv, Wp, bp):
    queries = np.asarray(queries, np.float32)
    keys = np.asarray(keys, np.float32)
    values = np.asarray(values, np.float32)
    Wq = np.ascontiguousarray(np.asarray(Wq, np.float32))
    Wk = np.ascontiguousarray(np.asarray(Wk, np.float32))
    Wv = np.ascontiguousarray(np.asarray(Wv, np.float32))
    bq = np.asarray(bq, np.float32).reshape(1, D)
    bk = np.asarray(bk, np.float32).reshape(1, D)
    bv = np.asarray(bv, np.float32).reshape(1, D)
    Wp = np.asarray(Wp, np.float32)
    bp = np.asarray(bp, np.float32)

    nc = _get_nc()
    qT = np.ascontiguousarray(queries.transpose(0, 2, 1))   # [B, D, L]
    kT = np.ascontiguousarray(keys.transpose(0, 2, 1))
    Wp_bf = np.asarray(Wp, dtype=ml_dtypes.bfloat16)
    in_maps = []
    for i in range(N_CORES):
        cols = slice(NSH * i, NSH * (i + 1))
        # bias pre-scrambled to the transposed-output layout [p, t, c, b]
        # (value at flat (t, p, c, b) = bp[8192 t + 128 c + p], replicated over b)
        bp_shard = np.asarray(bp[cols], np.float32).reshape(4, 64, 128)
        bp_scr = np.broadcast_to(
            bp_shard.transpose(2, 0, 1)[:, :, :, None], (128, 4, 64, 8))
        m = {
            "wq": Wq, "wk": Wk, "wv": Wv,
            "bq": bq, "bk": bk, "bv": bv,
            "wp": np.ascontiguousarray(Wp_bf[:, cols]),
            "bp": np.ascontiguousarray(
                np.asarray(bp_scr, dtype=ml_dtypes.bfloat16)).reshape(1, 4 * 128 * 64 * 8),
        }
        if MODE == "repl":
            m.update({"qt": qT.reshape(B * D, L), "kt": kT.reshape(B * D, L),
                      "v": values.reshape(B * L, D)})
        else:
            m.update({"qt": qT[i], "kt": kT[i], "v": values[i]})
        in_maps.append(m)
    res = run_bass_kernel_spmd(nc, in_maps, core_ids=list(range(N_CORES)), trace=TRACE)
    global LAST_RESULT
    LAST_RESULT = res
    shards = []
    for i in range(N_CORES):
        buf = np.asarray(res.results[i]["out"], np.float32)
        if buf.shape == (4 * 128, 64 * 8):
            # transposed layout: buf[(t p), (c b)] -> shard[b, 8192 t + 128 c + p]
            shards.append(buf.reshape(4, 128, 64, 8).transpose(3, 0, 2, 1).reshape(B, NSH))
        else:
            shards.append(buf)
    out = np.concatenate(shards, axis=1)
    return out.reshape(B, L, D)


# revision 13
# speedup vs baseline: 1.2721x; 1.2659x over previous
"""AutoCorrelation layer kernel for 8 Trainium2 NeuronCores.

Math note: the reference's rfft/irfft pair over the zero-padded head dim
computes a circular cross-correlation; its mean over all lags collapses
analytically to (sum_d q_proj) * (sum_d k_proj) per head.  So
corr_mean[b,l] = (1/(H*L)) * sum_h (q[b,l] @ WqS + bqS)_h * (k[b,l] @ WkS + bkS)_h
with WqS = Wq.reshape(D,H,DK).sum(-1).  Everything downstream (top-6,
softmax, gather, output projection) follows the reference directly.

Distribution (v2): batch-parallel preprocessing — core i computes the
corr/top-6/gather/agg for batch i only (f32 throughout: the 6th/7th
top-k gap can be ~1e-5 so bf16 corr flips selections), then a tiny
AllGather of the per-batch agg vectors [1,256] feeds the column-sharded
output projection.  Wp is pre-cast to bf16 on the host and streamed as
16 resident SBUF tiles whose DMAs all start at t=0; the big matmuls
consume them as soon as agg arrives.  v is fetched via a 6-row indexed
gather instead of a full 1MB load.
"""
import sys

sys.path.insert(0, "/opt/trn_rl_repo")

import numpy as np
import ml_dtypes
import concourse.bass as bass
import concourse.mybir as mybir
import concourse.tile as tile
from concourse import bacc
from concourse.bass_utils import run_bass_kernel_spmd
from concourse.masks import make_identity

F32 = mybir.dt.float32
BF16 = mybir.dt.bfloat16
U32 = mybir.dt.uint32

N_CORES = 8
B, L, D, H, DK = 8, 1024, 256, 8, 32
K_TOP = 6
NSH = (L * D) // N_CORES          # 32768 output cols per core
TILE_N = 2048
N_TILES = NSH // TILE_N           # 16
SUBS = TILE_N // 512              # 4
SCALE = 1.0 / (H * L)

TRACE = False          # test harness sets this for profiled runs
LAST_RESULT = None     # stashed BassKernelResults from the last kernel() call
MODE = "dp"            # "dp": batch-parallel preproc + AllGather; "repl": replicated preproc

_CACHE = {}


def _build_nc():
    nc = bacc.Bacc("TRN2", target_bir_lowering=False, debug=False, num_devices=N_CORES)

    qt_d = nc.dram_tensor("qt", [D, L], F32, kind="ExternalInput").ap()
    kt_d = nc.dram_tensor("kt", [D, L], F32, kind="ExternalInput").ap()
    v_d = nc.dram_tensor("v", [L, D], F32, kind="ExternalInput").ap()
    wq_d = nc.dram_tensor("wq", [D, D], F32, kind="ExternalInput").ap()
    wk_d = nc.dram_tensor("wk", [D, D], F32, kind="ExternalInput").ap()
    wv_d = nc.dram_tensor("wv", [D, D], F32, kind="ExternalInput").ap()
    bq_d = nc.dram_tensor("bq", [1, D], F32, kind="ExternalInput").ap()
    bk_d = nc.dram_tensor("bk", [1, D], F32, kind="ExternalInput").ap()
    bv_d = nc.dram_tensor("bv", [1, D], F32, kind="ExternalInput").ap()
    wp_d = nc.dram_tensor("wp", [D, NSH], BF16, kind="ExternalInput").ap()
    bp_d = nc.dram_tensor("bp", [1, 4 * 128 * 64 * 8], BF16, kind="ExternalInput").ap()
    out_d = nc.dram_tensor("out", [4 * 128, 64 * 8], BF16, kind="ExternalOutput").ap()

    with tile.TileContext(nc) as tc:
        with (
            tc.tile_pool(name="cst", bufs=1) as cst,
            tc.tile_pool(name="work", bufs=1) as work,
            tc.tile_pool(name="wpp", bufs=N_TILES) as wpp,
            tc.tile_pool(name="outp", bufs=2) as outp,
            tc.tile_pool(name="bpp", bufs=2) as bpp,
            tc.tile_pool(name="dr", bufs=1, space="DRAM") as dr,
            tc.tile_pool(name="ps_mm", bufs=1, space="PSUM") as ps_mm,
            tc.tile_pool(name="ps_tp", bufs=1, space="PSUM") as ps_tp,
            tc.tile_pool(name="ps_o", bufs=6, space="PSUM") as ps_o,
        ):
            # ---------------- phase 0: kick off all input DMAs ----------------
            # sync ring order: small weights -> qt/kt -> 16 wp tiles (16MB).
            wq_sb = cst.tile([128, 2, 256], F32)
            nc.sync.dma_start(wq_sb[:, :, :], wq_d.rearrange("(c p) d -> p c d", p=128))
            wk_sb = cst.tile([128, 2, 256], F32)
            nc.sync.dma_start(wk_sb[:, :, :], wk_d.rearrange("(c p) d -> p c d", p=128))
            wv_sb = cst.tile([128, 2, 256], F32)
            nc.sync.dma_start(wv_sb[:, :, :], wv_d.rearrange("(c p) d -> p c d", p=128))
            bq_sb = cst.tile([1, 256], F32)
            nc.sync.dma_start(bq_sb[:, :], bq_d)
            bk_sb = cst.tile([1, 256], F32)
            nc.sync.dma_start(bk_sb[:, :], bk_d)
            bv_sb = cst.tile([1, 256], F32)
            nc.sync.dma_start(bv_sb[:, :], bv_d)
            trq = work.tile([128, 2, L], F32)
            nc.sync.dma_start(trq[:, :, :], qt_d.rearrange("(c p) l -> p c l", p=128))
            trk = work.tile([128, 2, L], F32)
            nc.sync.dma_start(trk[:, :, :], kt_d.rearrange("(c p) l -> p c l", p=128))
            wpt = []
            for nt in range(N_TILES):
                ncol = slice(TILE_N * nt, TILE_N * (nt + 1))
                wp_t = wpp.tile([128, 2, TILE_N], BF16, tag="wp")
                eng = nc.sync if nt % 2 == 0 else nc.scalar
                eng.dma_start(
                    wp_t[:, :, :],
                    wp_d[:, ncol].rearrange("(c p) n -> p c n", p=128))
                wpt.append(wp_t)

            # ---------------- small constants ----------------
            ident8 = cst.tile([8, 8], F32)
            make_identity(nc, ident8[:, :])
            one1 = cst.tile([1, 1], F32)
            nc.vector.memset(one1[:, :], 1.0)
            sones = cst.tile([8, 1], F32)
            nc.vector.memset(sones[:, :], SCALE)

            # PE warm-up: the HAM clock gate needs ~3.4us of sustained PE
            # activity to lift the 1.2GHz cold throttle; burn it on junk
            # matmuls while the qt/kt DMAs are still in flight.
            ps_warm = ps_mm.tile([128, 512], F32, tag="mm")
            for _ in range(4):
                nc.tensor.matmul(ps_warm[:, 0:256], wq_sb[:, 0, 0:128], wq_sb[:, 0, :],
                                 start=True, stop=True)

            # head-sums of projection weights: WqS[d, h] = sum_z Wq[d, h*32+z]
            wqs = cst.tile([128, 2, 8], F32)
            nc.vector.reduce_sum(out=wqs[:, :, :],
                                 in_=wq_sb[:, :, :].rearrange("p c (h z) -> p c h z", z=DK),
                                 axis=mybir.AxisListType.X)
            wks = cst.tile([128, 2, 8], F32)
            nc.vector.reduce_sum(out=wks[:, :, :],
                                 in_=wk_sb[:, :, :].rearrange("p c (h z) -> p c h z", z=DK),
                                 axis=mybir.AxisListType.X)
            bqs_row = cst.tile([1, 8], F32)
            nc.vector.reduce_sum(out=bqs_row[:, :],
                                 in_=bq_sb[:, :].rearrange("o (h z) -> o h z", z=DK),
                                 axis=mybir.AxisListType.X)
            bks_row = cst.tile([1, 8], F32)
            nc.vector.reduce_sum(out=bks_row[:, :],
                                 in_=bk_sb[:, :].rearrange("o (h z) -> o h z", z=DK),
                                 axis=mybir.AxisListType.X)
            # [1,8] -> [8,1] via K=1 matmul against [1,1] ones
            bqs_ps = ps_tp.tile([8, 1], F32, tag="tp")
            nc.tensor.matmul(bqs_ps[:, :], bqs_row[:, :], one1[:, :], start=True, stop=True)
            bqs_vert = cst.tile([8, 1], F32)
            nc.vector.tensor_copy(bqs_vert[:, :], bqs_ps[:, :])
            bks_ps = ps_tp.tile([8, 1], F32, tag="tp")
            nc.tensor.matmul(bks_ps[:, :], bks_row[:, :], one1[:, :], start=True, stop=True)
            bks_vert = cst.tile([8, 1], F32)
            nc.vector.tensor_copy(bks_vert[:, :], bks_ps[:, :])

            # ---------------- per-batch corr (this core's batch only) ----------------
            xsT = {}
            for (tr, w_sum, bias_v, nm) in (
                (trq, wqs, bqs_vert, "q"),
                (trk, wks, bks_vert, "k"),
            ):
                xs = work.tile([8, L], F32, tag=f"{nm}sT")
                for half in range(2):
                    sl = slice(512 * half, 512 * (half + 1))
                    ps_x = ps_mm.tile([8, 512], F32, tag="mm")
                    nc.tensor.matmul(ps_x[:, :], w_sum[:, 0, :], tr[:, 0, sl], start=True, stop=False)
                    nc.tensor.matmul(ps_x[:, :], w_sum[:, 1, :], tr[:, 1, sl], start=False, stop=True)
                    nc.vector.tensor_scalar(
                        out=xs[:, sl], in0=ps_x[:, :],
                        scalar1=bias_v[:, 0:1], scalar2=None, op0=mybir.AluOpType.add)
                xsT[nm] = xs

            prod = work.tile([8, L], F32, tag="prod")
            nc.vector.tensor_mul(prod[:, :], xsT["q"][:, :], xsT["k"][:, :])
            r_sb = work.tile([1, L], F32, tag="qsT")
            for half in range(2):
                sl = slice(512 * half, 512 * (half + 1))
                ps_r = ps_mm.tile([1, 512], F32, tag="mm")
                nc.tensor.matmul(ps_r[:, :], sones[:, :], prod[:, sl], start=True, stop=True)
                nc.vector.tensor_copy(r_sb[:, sl], ps_r[:, :])

            # ---------------- top-6, softmax, 6-row gather of v ----------------
            topv = work.tile([1, 8], F32)
            nc.vector.max(topv[:, :], r_sb[:, :])
            topi = work.tile([1, 8], U32)
            nc.vector.max_index(topi[:, :], topv[:, :], r_sb[:, :])
            negm0 = work.tile([1, 1], F32)
            nc.vector.tensor_scalar_mul(negm0[:, :], topv[:, 0:1], -1.0)
            e_sb = work.tile([1, K_TOP], F32)
            nc.scalar.activation(e_sb[:, :], topv[:, 0:K_TOP],
                                 mybir.ActivationFunctionType.Exp,
                                 bias=negm0[:, 0:1], scale=1.0)
            z_sb = work.tile([1, 1], F32)
            nc.vector.reduce_sum(out=z_sb[:, :], in_=e_sb[:, :], axis=mybir.AxisListType.X)
            zinv = work.tile([1, 1], F32)
            nc.vector.reciprocal(zinv[:, :], z_sb[:, :])
            w_sb = work.tile([1, K_TOP], F32)
            nc.vector.tensor_scalar_mul(w_sb[:, :], e_sb[:, :], zinv[:, 0:1])

            # indices/weights -> columns via K=1 matmuls
            topi_f = work.tile([1, 8], F32)
            nc.vector.tensor_copy(topi_f[:, :], topi[:, :])
            idx_ps = ps_tp.tile([8, 1], F32, tag="tp")
            nc.tensor.matmul(idx_ps[:, :], topi_f[:, :], one1[:, :], start=True, stop=True)
            idx_colf = work.tile([8, 1], F32)
            nc.vector.tensor_copy(idx_colf[:, :], idx_ps[:, :])
            idx_col = work.tile([8, 1], U32)
            nc.vector.tensor_copy(idx_col[:, :], idx_colf[:, :])
            w_ps = ps_tp.tile([K_TOP, 1], F32, tag="tp")
            nc.tensor.matmul(w_ps[:, :], w_sb[:, :], one1[:, :], start=True, stop=True)
            w_col = work.tile([K_TOP, 1], F32)
            nc.vector.tensor_copy(w_col[:, :], w_ps[:, :])

            vrows = work.tile([K_TOP, 256], F32)
            nc.gpsimd.indirect_dma_start(
                out=vrows[:, :],
                out_offset=None,
                in_=v_d[:, :],
                in_offset=bass.IndirectOffsetOnAxis(ap=idx_col[0:K_TOP, 0:1], axis=0),
            )

            # vbar[1, 256] = w^T @ vrows
            vb_ps = ps_tp.tile([1, 256], F32, tag="tp")
            nc.tensor.matmul(vb_ps[:, :], w_col[:, :], vrows[:, :], start=True, stop=True)
            vbar = work.tile([1, 256], F32)
            nc.vector.tensor_copy(vbar[:, :], vb_ps[:, :])
            # vbar -> [128, 2] column chunks
            vbarT = work.tile([128, 2], F32)
            for m in range(2):
                pv = ps_tp.tile([128, 1], F32, tag="tp")
                nc.tensor.matmul(pv[:, :], vbar[0:1, 128 * m:128 * (m + 1)], one1[:, :],
                                 start=True, stop=True)
                nc.vector.tensor_copy(vbarT[:, m:m + 1], pv[:, :])

            # agg[d'] = sum_e Wv[e, d'] vbar[e] + bv[d']  -> [128, 2] (d' chunks)
            agg_sb = work.tile([128, 2], F32)
            for m in range(2):
                pa = ps_tp.tile([128, 1], F32, tag="tp")
                nc.tensor.matmul(pa[:, :], wv_sb[:, 0, 128 * m:128 * (m + 1)],
                                 vbarT[:, 0:1], start=True, stop=False)
                nc.tensor.matmul(pa[:, :], wv_sb[:, 1, 128 * m:128 * (m + 1)],
                                 vbarT[:, 1:2], start=False, stop=False)
                nc.tensor.matmul(pa[:, :], bv_sb[0:1, 128 * m:128 * (m + 1)],
                                 one1[:, :], start=False, stop=True)
                nc.vector.tensor_copy(agg_sb[:, m:m + 1], pa[:, :])

            # ---------------- AllGather agg -> [8, 256] ----------------
            agg_in = dr.tile([1, D], F32)
            nc.gpsimd.dma_start(
                agg_in[:, :].rearrange("o (m e) -> (o e) m", e=128), agg_sb[:, :])
            agg_out = dr.tile([B, D], F32)
            nc.gpsimd.collective_compute(
                "AllGather", mybir.AluOpType.bypass,
                replica_groups=[list(range(N_CORES))],
                ins=[agg_in[:, :].opt()], outs=[agg_out[:, :].opt()])
            aggf = cst.tile([8, 256], F32)
            nc.gpsimd.dma_start(aggf[:, :], agg_out[:, :])
            aggt_bf = cst.tile([128, 16], BF16)
            for m in range(2):
                pt = ps_tp.tile([128, 8], F32, tag="tp")
                nc.tensor.transpose(pt[:, :], aggf[0:8, 128 * m:128 * (m + 1)], ident8[:, :])
                nc.vector.tensor_copy(aggt_bf[:, 8 * m:8 * (m + 1)], pt[:, :])

            # ---------------- big output projection, transposed ----------------
            # outT[n, b] = sum_k Wp[k, n] agg[b, k]: Wp chunks are the
            # STATIONARY operand (M=128), aggt streams (N=8).  PSUM fills a
            # whole bank [128, 64, 8] before one full-width DVE drain that
            # also adds the (host-scrambled, pre-replicated) bias.  Host
            # unscrambles the [4, 128, 64, 8] output layout.
            bp_sb = cst.tile([128, 4, 64, 8], BF16)
            nc.sync.dma_start(bp_sb[:, :, :, :], bp_d.rearrange("o (p t c b) -> (o p) t c b", t=4, p=128, c=64))
            for t in range(4):
                ps = ps_o.tile([128, 64, 8], F32, tag="po")
                for c in range(64):
                    n0 = 8192 * t + 128 * c
                    wt = wpt[n0 // TILE_N]
                    co = n0 % TILE_N
                    nc.tensor.matmul(ps[:, c, :], wt[:, 0, co:co + 128], aggt_bf[:, 0:8],
                                     start=True, stop=False)
                    nc.tensor.matmul(ps[:, c, :], wt[:, 1, co:co + 128], aggt_bf[:, 8:16],
                                     start=False, stop=True)
                o_sbT = outp.tile([128, 64, 8], BF16)
                nc.vector.tensor_add(o_sbT[:, :, :], ps[:, :, :], bp_sb[:, t, :, :])
                nc.gpsimd.dma_start(out_d[128 * t:128 * (t + 1), :], o_sbT[:, :, :].rearrange("p c b -> p (c b)"))

    nc.finalize()
    return nc


def _build_nc_repl():
    """Replicated preprocessing: every core computes corr/top-6/agg for ALL
    8 batches (16MB redundant qt/kt read) so no cross-core collective is
    needed; the column-sharded projection starts as soon as local preproc
    finishes (~65us) instead of waiting ~90us for the AllGather."""
    nc = bacc.Bacc("TRN2", target_bir_lowering=False, debug=False, num_devices=N_CORES)

    qt_d = nc.dram_tensor("qt", [B * D, L], F32, kind="ExternalInput").ap()
    kt_d = nc.dram_tensor("kt", [B * D, L], F32, kind="ExternalInput").ap()
    v_d = nc.dram_tensor("v", [B * L, D], F32, kind="ExternalInput").ap()
    wq_d = nc.dram_tensor("wq", [D, D], F32, kind="ExternalInput").ap()
    wk_d = nc.dram_tensor("wk", [D, D], F32, kind="ExternalInput").ap()
    wv_d = nc.dram_tensor("wv", [D, D], F32, kind="ExternalInput").ap()
    bq_d = nc.dram_tensor("bq", [1, D], F32, kind="ExternalInput").ap()
    bk_d = nc.dram_tensor("bk", [1, D], F32, kind="ExternalInput").ap()
    bv_d = nc.dram_tensor("bv", [1, D], F32, kind="ExternalInput").ap()
    wp_d = nc.dram_tensor("wp", [D, NSH], BF16, kind="ExternalInput").ap()
    bp_d = nc.dram_tensor("bp", [1, 4 * 128 * 64 * 8], BF16, kind="ExternalInput").ap()
    out_d = nc.dram_tensor("out", [4 * 128, 64 * 8], BF16, kind="ExternalOutput").ap()

    with tile.TileContext(nc) as tc:
        with (
            tc.tile_pool(name="cst", bufs=1) as cst,
            tc.tile_pool(name="work", bufs=1) as work,
            tc.tile_pool(name="trp", bufs=3) as trp,
            tc.tile_pool(name="wpp", bufs=N_TILES) as wpp,
            tc.tile_pool(name="outp", bufs=2) as outp,
            tc.tile_pool(name="bpp", bufs=2) as bpp,
            tc.tile_pool(name="ps_mm", bufs=1, space="PSUM") as ps_mm,
            tc.tile_pool(name="ps_tp", bufs=1, space="PSUM") as ps_tp,
            tc.tile_pool(name="ps_o", bufs=6, space="PSUM") as ps_o,
        ):
            # ---- phase 0: all input DMAs; qt/kt tiles split across both
            # HWDGE rings ahead of the wp stream ----
            wq_sb = cst.tile([128, 2, 256], F32)
            nc.sync.dma_start(wq_sb[:, :, :], wq_d.rearrange("(c p) d -> p c d", p=128))
            wk_sb = cst.tile([128, 2, 256], F32)
            nc.sync.dma_start(wk_sb[:, :, :], wk_d.rearrange("(c p) d -> p c d", p=128))
            wv_sb = cst.tile([128, 2, 256], F32)
            nc.sync.dma_start(wv_sb[:, :, :], wv_d.rearrange("(c p) d -> p c d", p=128))
            bq_sb = cst.tile([1, 256], F32)
            nc.sync.dma_start(bq_sb[:, :], bq_d)
            bk_sb = cst.tile([1, 256], F32)
            nc.sync.dma_start(bk_sb[:, :], bk_d)
            bv_sb = cst.tile([1, 256], F32)
            nc.sync.dma_start(bv_sb[:, :], bv_d)
            # per-batch transposed q/k tiles: sync carries q, scalar carries k
            trqs, trks = [], []
            for b in range(B):
                trq = trp.tile([128, 2, L], F32, tag="trq")
                nc.sync.dma_start(
                    trq[:, :, :],
                    qt_d[D * b:D * (b + 1), :].rearrange("(c p) l -> p c l", p=128))
                trqs.append(trq)
                trk = trp.tile([128, 2, L], F32, tag="trk")
                nc.scalar.dma_start(
                    trk[:, :, :],
                    kt_d[D * b:D * (b + 1), :].rearrange("(c p) l -> p c l", p=128))
                trks.append(trk)
            wpt = []
            for nt in range(N_TILES):
                ncol = slice(TILE_N * nt, TILE_N * (nt + 1))
                wp_t = wpp.tile([128, 2, TILE_N], BF16, tag="wp")
                eng = nc.sync if nt % 2 == 0 else nc.scalar
                eng.dma_start(
                    wp_t[:, :, :],
                    wp_d[:, ncol].rearrange("(c p) n -> p c n", p=128))
                wpt.append(wp_t)

            # ---- small constants ----
            ident8 = cst.tile([8, 8], F32)
            make_identity(nc, ident8[:, :])
            one1 = cst.tile([1, 1], F32)
            nc.vector.memset(one1[:, :], 1.0)
            sones = cst.tile([8, 1], F32)
            nc.vector.memset(sones[:, :], SCALE)

            # PE warm-up while the first qt/kt tiles are in flight
            ps_warm = ps_mm.tile([128, 512], F32, tag="mm")
            for _ in range(4):
                nc.tensor.matmul(ps_warm[:, 0:256], wq_sb[:, 0, 0:128], wq_sb[:, 0, :],
                                 start=True, stop=True)

            wqs = cst.tile([128, 2, 8], F32)
            nc.vector.reduce_sum(out=wqs[:, :, :],
                                 in_=wq_sb[:, :, :].rearrange("p c (h z) -> p c h z", z=DK),
                                 axis=mybir.AxisListType.X)
            wks = cst.tile([128, 2, 8], F32)
            nc.vector.reduce_sum(out=wks[:, :, :],
                                 in_=wk_sb[:, :, :].rearrange("p c (h z) -> p c h z", z=DK),
                                 axis=mybir.AxisListType.X)
            bqs_row = cst.tile([1, 8], F32)
            nc.vector.reduce_sum(out=bqs_row[:, :],
                                 in_=bq_sb[:, :].rearrange("o (h z) -> o h z", z=DK),
                                 axis=mybir.AxisListType.X)
            bks_row = cst.tile([1, 8], F32)
            nc.vector.reduce_sum(out=bks_row[:, :],
                                 in_=bk_sb[:, :].rearrange("o (h z) -> o h z", z=DK),
                                 axis=mybir.AxisListType.X)
            bqs_ps = ps_tp.tile([8, 1], F32, tag="tp")
            nc.tensor.matmul(bqs_ps[:, :], bqs_row[:, :], one1[:, :], start=True, stop=True)
            bqs_vert = cst.tile([8, 1], F32)
            nc.vector.tensor_copy(bqs_vert[:, :], bqs_ps[:, :])
            bks_ps = ps_tp.tile([8, 1], F32, tag="tp")
            nc.tensor.matmul(bks_ps[:, :], bks_row[:, :], one1[:, :], start=True, stop=True)
            bks_vert = cst.tile([8, 1], F32)
            nc.vector.tensor_copy(bks_vert[:, :], bks_ps[:, :])

            # ---- per-batch corr, top-6, softmax; collect idx/w columns ----
            idx48 = work.tile([48, 1], F32)
            wmat = work.tile([48, 8], F32)
            nc.vector.memset(wmat[:, :], 0.0)
            for b in range(B):
                xsT = {}
                for (tr, w_sum, bias_v, nm) in (
                    (trqs[b], wqs, bqs_vert, "q"),
                    (trks[b], wks, bks_vert, "k"),
                ):
                    xs = work.tile([8, L], F32, tag=f"{nm}sT")
                    for half in range(2):
                        sl = slice(512 * half, 512 * (half + 1))
                        ps_x = ps_mm.tile([8, 512], F32, tag="mm")
                        nc.tensor.matmul(ps_x[:, :], w_sum[:, 0, :], tr[:, 0, sl], start=True, stop=False)
                        nc.tensor.matmul(ps_x[:, :], w_sum[:, 1, :], tr[:, 1, sl], start=False, stop=True)
                        nc.vector.tensor_scalar(
                            out=xs[:, sl], in0=ps_x[:, :],
                            scalar1=bias_v[:, 0:1], scalar2=None, op0=mybir.AluOpType.add)
                    xsT[nm] = xs
                prod = work.tile([8, L], F32, tag="prod")
                nc.vector.tensor_mul(prod[:, :], xsT["q"][:, :], xsT["k"][:, :])
                r_sb = work.tile([1, L], F32, tag="qsT")
                for half in range(2):
                    sl = slice(512 * half, 512 * (half + 1))
                    ps_r = ps_mm.tile([1, 512], F32, tag="mm")
                    nc.tensor.matmul(ps_r[:, :], sones[:, :], prod[:, sl], start=True, stop=True)
                    nc.vector.tensor_copy(r_sb[:, sl], ps_r[:, :])

                topv = work.tile([1, 8], F32, tag="topv")
                nc.vector.max(topv[:, :], r_sb[:, :])
                topi = work.tile([1, 8], U32, tag="topi")
                nc.vector.max_index(topi[:, :], topv[:, :], r_sb[:, :])
                negm0 = work.tile([1, 1], F32, tag="negm0")
                nc.vector.tensor_scalar_mul(negm0[:, :], topv[:, 0:1], -1.0)
                e_sb = work.tile([1, K_TOP], F32, tag="e_sb")
                nc.scalar.activation(e_sb[:, :], topv[:, 0:K_TOP],
                                     mybir.ActivationFunctionType.Exp,
                                     bias=negm0[:, 0:1], scale=1.0)
                z_sb = work.tile([1, 1], F32, tag="z_sb")
                nc.vector.reduce_sum(out=z_sb[:, :], in_=e_sb[:, :], axis=mybir.AxisListType.X)
                zinv = work.tile([1, 1], F32, tag="zinv")
                nc.vector.reciprocal(zinv[:, :], z_sb[:, :])
                w_sb = work.tile([1, K_TOP], F32, tag="w_sb")
                nc.vector.tensor_scalar_mul(w_sb[:, :], e_sb[:, :], zinv[:, 0:1])

                # indices (+1024*b) and weights -> columns of idx48 / wmat
                topi_f = work.tile([1, 8], F32, tag="topi_f")
                nc.vector.tensor_copy(topi_f[:, :], topi[:, :])
                idx_ps = ps_tp.tile([8, 1], F32, tag="tp")
                nc.tensor.matmul(idx_ps[:, :], topi_f[:, :], one1[:, :], start=True, stop=True)
                nc.vector.tensor_scalar(
                    out=idx48[6 * b:6 * (b + 1), 0:1], in0=idx_ps[0:K_TOP, :],
                    scalar1=None, scalar2=None, op0=mybir.AluOpType.add,
                    const1=float(L * b))
                w_ps = ps_tp.tile([K_TOP, 1], F32, tag="tp")
                nc.tensor.matmul(w_ps[:, :], w_sb[:, :], one1[:, :], start=True, stop=True)
                nc.vector.tensor_copy(wmat[6 * b:6 * (b + 1), b:b + 1], w_ps[:, :])

            # ---- one 48-row gather of v, batched weighted-sum, Wv proj ----
            idx_u = work.tile([48, 1], U32)
            nc.vector.tensor_copy(idx_u[:, :], idx48[:, :])
            vrows = work.tile([48, 256], F32)
            nc.gpsimd.indirect_dma_start(
                out=vrows[:, :],
                out_offset=None,
                in_=v_d[:, :],
                in_offset=bass.IndirectOffsetOnAxis(ap=idx_u[:, 0:1], axis=0),
            )
            # vbar[8, 256] = wmat.T @ vrows  (block-diagonal weights)
            ps_vb = ps_mm.tile([8, 512], F32, tag="mm")
            nc.tensor.matmul(ps_vb[:, 0:256], wmat[:, :], vrows[:, :], start=True, stop=True)
            vbar = work.tile([8, 256], F32)
            nc.vector.tensor_copy(vbar[:, :], ps_vb[:, 0:256])
            # vbarT [128, 2, 8]
            vbarT = work.tile([128, 2, 8], F32)
            for m in range(2):
                pt = ps_tp.tile([128, 8], F32, tag="tp")
                nc.tensor.transpose(pt[:, :], vbar[0:8, 128 * m:128 * (m + 1)], ident8[:, :])
                nc.vector.tensor_copy(vbarT[:, m, :], pt[:, :])
            # aggf[8, 256] = vbar @ Wv + bv
            ps_a = ps_mm.tile([8, 512], F32, tag="mm")
            nc.tensor.matmul(ps_a[:, 0:256], vbarT[:, 0, :], wv_sb[:, 0, :], start=True, stop=False)
            nc.tensor.matmul(ps_a[:, 0:256], vbarT[:, 1, :], wv_sb[:, 1, :], start=False, stop=False)
            ones8m = cst.tile([1, 8], F32)
            nc.vector.memset(ones8m[:, :], 1.0)
            nc.tensor.matmul(ps_a[:, 0:256], ones8m[:, :], bv_sb[:, :], start=False, stop=True)
            aggf = cst.tile([8, 256], F32)
            nc.vector.tensor_copy(aggf[:, :], ps_a[:, 0:256])
            aggt_bf = cst.tile([128, 16], BF16)
            for m in range(2):
                pt2 = ps_tp.tile([128, 8], F32, tag="tp")
                nc.tensor.transpose(pt2[:, :], aggf[0:8, 128 * m:128 * (m + 1)], ident8[:, :])
                nc.vector.tensor_copy(aggt_bf[:, 8 * m:8 * (m + 1)], pt2[:, :])

            # ---- big output projection, transposed (see dp build) ----
            bp_sb = cst.tile([128, 4, 64, 8], BF16)
            nc.sync.dma_start(bp_sb[:, :, :, :], bp_d.rearrange("o (p t c b) -> (o p) t c b", t=4, p=128, c=64))
            for t in range(4):
                ps = ps_o.tile([128, 64, 8], F32, tag="po")
                for c in range(64):
                    n0 = 8192 * t + 128 * c
                    wt = wpt[n0 // TILE_N]
                    co = n0 % TILE_N
                    nc.tensor.matmul(ps[:, c, :], wt[:, 0, co:co + 128], aggt_bf[:, 0:8],
                                     start=True, stop=False)
                    nc.tensor.matmul(ps[:, c, :], wt[:, 1, co:co + 128], aggt_bf[:, 8:16],
                                     start=False, stop=True)
                o_sbT = outp.tile([128, 64, 8], BF16)
                nc.vector.tensor_add(o_sbT[:, :, :], ps[:, :, :], bp_sb[:, t, :, :])
                nc.gpsimd.dma_start(out_d[128 * t:128 * (t + 1), :], o_sbT[:, :, :].rearrange("p c b -> p (c b)"))

    nc.finalize()
    return nc


def _get_nc():
    if "nc" not in _CACHE:
        _CACHE["nc"] = _build_nc_repl() if MODE == "repl" else _build_nc()
    return _CACHE["nc"]


def kernel(queries, keys, values, Wq, bq, Wk, bk, Wv, bv, Wp, bp):
    queries = np.asarray(queries, np.float32)
    keys = np.asarray(keys, np.float32)
    values = np.asarray(values, np.float32)
    Wq = np.ascontiguousarray(np.asarray(Wq, np.float32))
    Wk = np.ascontiguousarray(np.asarray(Wk, np.float32))
    Wv = np.ascontiguousarray(np.asarray(Wv, np.float32))
    bq = np.asarray(bq, np.float32).reshape(1, D)
    bk = np.asarray(bk, np.float32).reshape(1, D)
    bv = np.asarray(bv, np.float32).reshape(1, D)
    Wp = np.asarray(Wp, np.float32)
    bp = np.asarray(bp, np.float32)

    nc = _get_nc()
    qT = np.ascontiguousarray(queries.transpose(0, 2, 1))   # [B, D, L]
    kT = np.ascontiguousarray(keys.transpose(0, 2, 1))
    Wp_bf = np.asarray(Wp, dtype=ml_dtypes.bfloat16)
    in_maps = []
    for i in range(N_CORES):
        cols = slice(NSH * i, NSH * (i + 1))
        # bias pre-scrambled to the transposed-output layout [p, t, c, b]
        # (value at flat (t, p, c, b) = bp[8192 t + 128 c + p], replicated over b)
        bp_shard = np.asarray(bp[cols], np.float32).reshape(4, 64, 128)
        bp_scr = np.broadcast_to(
            bp_shard.transpose(2, 0, 1)[:, :, :, None], (128, 4, 64, 8))
        m = {
            "wq": Wq, "wk": Wk, "wv": Wv,
            "bq": bq, "bk": bk, "bv": bv,
            "wp": np.ascontiguousarray(Wp_bf[:, cols]),
            "bp": np.ascontiguousarray(
                np.asarray(bp_scr, dtype=ml_dtypes.bfloat16)).reshape(1, 4 * 128 * 64 * 8),
        }
        if MODE == "repl":
            m.update({"qt": qT.reshape(B * D, L), "kt": kT.reshape(B * D, L),
                      "v": values.reshape(B * L, D)})
        else:
            m.update({"qt": qT[i], "kt": kT[i], "v": values[i]})
        in_maps.append(m)
    res = run_bass_kernel_spmd(nc, in_maps, core_ids=list(range(N_CORES)), trace=TRACE)
    global LAST_RESULT
    LAST_RESULT = res
    shards = []
    for i in range(N_CORES):
        buf = np.asarray(res.results[i]["out"], np.float32)
        if buf.shape == (4 * 128, 64 * 8):
            # transposed layout: buf[(t p), (c b)] -> shard[b, 8192 t + 128 c + p]
            shards.append(buf.reshape(4, 128, 64, 8).transpose(3, 0, 2, 1).reshape(B, NSH))
        else:
            shards.append(buf)
    out = np.concatenate(shards, axis=1)
    return out.reshape(B, L, D)
